# revision 1
# baseline (speedup 1.0000x reference)
"""Performer attention (causal, kernelized) — Trainium2 Bass kernel, v3.

Two launches on 8 cores:

  A) seq-sharded prep: core j owns 256 sequence positions and computes, for
     ALL 8 heads at once: kh (scaled k-projection), the LayerNorm-folded and
     scaled/biased q-projection qh, the v-projection in seq-major layout,
     and the local stabilizer max(h_k).  Each (position, head) projection is
     computed exactly once fleet-wide.

  B) head-sharded attention: core h owns head h end-to-end: Performer
     feature maps, the causal chunked prefix scan, output normalization and
     its row-block of the FC (W_fc row-sharded; host sums partials and adds
     bias + residual).  The exact global k_stab (host max over the 8 phase-A
     stabs) is folded into the k-feature exp bias — no approximation.

All big matmuls use float32r (4x PE throughput at free >= 256, ~2e-4
relative error; end-to-end max-rel stays ~1e-3).  Algebra notes (validated
against the reference):
  - q LayerNorm folded: Wq_eff = diag(gamma) Wq * scale, bias cq = beta@Wq*scale,
    applied to (q - mu) * rstd with rstd = exp(-0.5 ln(var + eps)).
  - exp(h_q + (proj_q - h_q)) == exp(proj_q): q-side stabilizer cancels.
  - k feature: exp(proj_k + h_k - k_stab) via the augmented contraction
    [kh; kh^2] . [rf^T; -0.5] plus a per-partition bias of -k_stab in the exp.
  - +KERNEL_EPS becomes extra features: q~ rows 266/267 = (sum_m exp_q + m*eps,
    eps); k~ cols 266/267 = (eps, sum_m exp_k); the global 1/sqrt(m) cancels
    except 1/c^2 folded into W_fc.
  - causal prefix scan chunked at C=128 with states per chunk-PAIR: the
    in-pair cross term (keys of even chunk x queries of odd chunk) rides in a
    [128 x 256] pair-attention block at full fp32r speed.
  - the non-causal normalizer d = q~ . z (z = column sums of k~) is computed
    from a separate z accumulation: o columns 64/66 carry D and d; no state
    column fixups needed.
  - the reference's |d|<=1e-6 guard is dead for any realistic data (d ~ 1e3+)
    and is omitted.
"""

import sys
for _p in ("/opt/trn_rl_repo", "/root/.axon_site/_ro/trn_rl_repo"):
    if _p not in sys.path:
        sys.path.append(_p)

import numpy as np

import concourse.bass as bass
from concourse import bacc
import concourse.mybir as mybir
import concourse.tile as tile
from concourse.bass import ts, ds
from concourse.bass_utils import run_bass_kernel_spmd

F32 = mybir.dt.float32
F32R = mybir.dt.float32r
NC = 8
N = 2048
D_MODEL = 512
D_K = 64
D_V = 64
M = 266
C = 128
NCH = N // C            # 16 chunks
NPAIR = NCH // 2        # 8 chunk pairs
SLA = N // NC           # 256 seq positions per phase-A core
NSL = 4                 # 512-wide slices of the full sequence
SL = 512
KERNEL_EPS = 1e-4
LN_EPS = 1e-6
SCALE = float(D_MODEL) ** (-0.25)
EXP = mybir.ActivationFunctionType.Exp
LN_F = mybir.ActivationFunctionType.Ln
IDENT = mybir.ActivationFunctionType.Identity


# --------------------------------------------------------------------------
# Phase A: seq-sharded projections + local stabilizer
# --------------------------------------------------------------------------
def build_phase_a():
    nc = bacc.Bacc("TRN2", target_bir_lowering=False, debug=False, num_devices=NC)
    xs = nc.dram_tensor("xs", [D_MODEL, 3 * SLA], F32, kind="ExternalInput")
    Wqe = nc.dram_tensor("Wqe", [D_MODEL, D_MODEL], F32, kind="ExternalInput")
    Wke = nc.dram_tensor("Wke", [D_MODEL, D_MODEL], F32, kind="ExternalInput")
    Wv = nc.dram_tensor("Wv", [D_MODEL, D_MODEL], F32, kind="ExternalInput")
    cq = nc.dram_tensor("cq", [128, 4], F32, kind="ExternalInput")
    wmean = nc.dram_tensor("wmean", [128, 1], F32, kind="ExternalInput")
    nh2 = nc.dram_tensor("nh2", [128, 2], F32, kind="ExternalInput")
    ones_r = nc.dram_tensor("ones_r", [1, 128], F32, kind="ExternalInput")
    gqneg = nc.dram_tensor("gqneg", [1, D_MODEL], F32, kind="ExternalInput")
    ident2 = nc.dram_tensor("ident2", [2, 2], F32, kind="ExternalInput")
    misc = nc.dram_tensor("misc", [1, 1], F32, kind="ExternalInput")  # LN_EPS
    kh_out = nc.dram_tensor("kh", [D_MODEL, SLA], F32, kind="ExternalOutput")
    qh_out = nc.dram_tensor("qh", [D_MODEL, SLA], F32, kind="ExternalOutput")
    vhT_out = nc.dram_tensor("vhT", [SLA, D_MODEL], F32, kind="ExternalOutput")
    stab_out = nc.dram_tensor("stab", [1, 1], F32, kind="ExternalOutput")

    with tile.TileContext(nc) as tc:
        with (
            tc.tile_pool(name="wts", bufs=1) as wts,
            tc.tile_pool(name="xin", bufs=1) as xin,
            tc.tile_pool(name="work", bufs=1) as work,
            tc.tile_pool(name="stat", bufs=1) as statp,
            tc.tile_pool(name="outs", bufs=1) as outs,
        ):
            # ---- loads; order chosen so compute can start early:
            # x (q/k/v slices) first -> LN stats chain; then Wk -> kh; Wq; Wv.
            q_r = xin.tile([128, 4, SLA], F32R)
            nc.gpsimd.dma_start(out=q_r, in_=xs[:, 0:SLA].rearrange("(c p) f -> p c f", p=128))
            k_r = xin.tile([128, 4, SLA], F32R)
            nc.gpsimd.dma_start(out=k_r, in_=xs[:, SLA:2 * SLA].rearrange("(c p) f -> p c f", p=128))
            v_r = xin.tile([128, 4, SLA], F32R)
            nc.gpsimd.dma_start(out=v_r, in_=xs[:, 2 * SLA:3 * SLA].rearrange("(c p) f -> p c f", p=128))
            wk_r = wts.tile([128, 4, D_MODEL], F32R)
            nc.gpsimd.dma_start(out=wk_r, in_=Wke[:, :].rearrange("(c p) f -> p c f", p=128))
            wv_r = wts.tile([128, 4, D_MODEL], F32R)
            nc.gpsimd.dma_start(out=wv_r, in_=Wv[:, :].rearrange("(c p) f -> p c f", p=128))
            wq_r = wts.tile([128, 4, D_MODEL], F32R)
            nc.gpsimd.dma_start(out=wq_r, in_=Wqe[:, :].rearrange("(c p) f -> p c f", p=128))
            wm_f = wts.tile([128, 1], F32)
            nc.sync.dma_start(out=wm_f, in_=wmean[:, :])
            wm_r = wts.tile([128, 1], F32R)
            nc.vector.tensor_copy(wm_r, wm_f)
            nh2_f = wts.tile([128, 2], F32)
            nc.sync.dma_start(out=nh2_f, in_=nh2[:, :])
            nh2_r = wts.tile([128, 2], F32R)
            nc.vector.tensor_copy(nh2_r, nh2_f)
            on_f = wts.tile([1, 128], F32)
            nc.sync.dma_start(out=on_f, in_=ones_r[:, :])
            on_r = wts.tile([1, 128], F32R)
            nc.vector.tensor_copy(on_r, on_f)
            gq_f = wts.tile([1, D_MODEL], F32)
            nc.sync.dma_start(out=gq_f, in_=gqneg[:, :])
            gq_r = wts.tile([1, D_MODEL], F32R)
            nc.vector.tensor_copy(gq_r, gq_f)
            id2_f = wts.tile([2, 2], F32)
            nc.sync.dma_start(out=id2_f, in_=ident2[:, :])
            cq_sb = wts.tile([128, 4], F32)
            nc.sync.dma_start(out=cq_sb, in_=cq[:, :])
            misc_sb = wts.tile([1, 1], F32)
            nc.sync.dma_start(out=misc_sb, in_=misc[:, :])
            warm = statp.tile([1, 1], F32)
            nc.scalar.activation(warm, misc_sb, LN_F, bias=1.0, scale=1.0)
            nc.scalar.activation(warm, warm, EXP, bias=0.0, scale=0.0)

            def q_c(c):
                return q_r[:, c, :]

            def k_c(c):
                return k_r[:, c, :]

            def v_c(c):
                return v_r[:, c, :]

            # ---- LayerNorm stats on q (over d_model, per position).
            # LN is folded into the projection: qh = rstd*(Wq_eff^T q - gq*mu)
            # + cq, so the q-projection itself never waits on this chain.
            mu_r = statp.tile([1, SLA], F32R)
            rsbc_r = work.tile([128, SLA], F32)
            with (
                tc.tile_pool(name="pss", bufs=1, space="PSUM") as pss,
                tc.tile_pool(name="psr", bufs=1, space="PSUM") as psr,
            ):
                mu_ps = pss.tile([1, SLA], F32, tag="mu")
                for c in range(4):
                    nc.tensor.matmul(mu_ps, wm_r, q_c(c), start=(c == 0),
                                     stop=(c == 3), skip_group_check=True)
                qsq_r = work.tile([128, 4, SLA], F32R)
                for c in range(4):
                    nc.vector.tensor_mul(qsq_r[:, c, :], q_c(c), q_c(c))
                msq_ps = pss.tile([1, SLA], F32, tag="msq")
                for c in range(4):
                    nc.tensor.matmul(msq_ps, wm_r, qsq_r[:, c, :], start=(c == 0),
                                     stop=(c == 3), skip_group_check=True)
                nc.vector.tensor_copy(mu_r, mu_ps)
                var_sb = statp.tile([1, SLA], F32)
                nc.vector.tensor_mul(var_sb, mu_r, mu_r)
                nc.vector.tensor_sub(var_sb, msq_ps, var_sb)
                rstd_r = statp.tile([1, SLA], F32R)
                nc.scalar.activation(rstd_r, var_sb, LN_F,
                                     bias=misc_sb[0:1, 0:1], scale=1.0)
                nc.scalar.activation(rstd_r, rstd_r, EXP, bias=0.0, scale=-0.5)

            # ---- projections: kh first (only needs Wk), then qh, then vh ----
            kh_sb = outs.tile([128, 4, SLA], F32)
            kh2_r = work.tile([128, 4, SLA], F32R)
            qh_sb = outs.tile([128, 4, SLA], F32)
            vhT_sb = outs.tile([128, 2, D_MODEL], F32)
            with tc.tile_pool(name="psb", bufs=2, space="PSUM") as psb:
                for oc in range(4):
                    kh_ps = psb.tile([128, SLA], F32, tag="kh")
                    for c in range(4):
                        nc.tensor.matmul(kh_ps, wk_r[:, c, ts(oc, 128)],
                                         k_c(c), start=(c == 0),
                                         stop=(c == 3), skip_group_check=True)
                    nc.scalar.copy(kh_sb[:, oc, :], kh_ps)
                    nc.vector.tensor_mul(kh2_r[:, oc, :], kh_sb[:, oc, :],
                                         kh_sb[:, oc, :])
                nc.sync.dma_start(
                    out=kh_out[:, :].rearrange("(c p) f -> p c f", p=128),
                    in_=kh_sb)

                # local stabilizer from kh^2 (small; overlaps Wq/Wv loads)
                hkm = statp.tile([2, 4], F32)
                for oc in range(4):
                    hk_ps = psb.tile([2, SLA], F32, tag="hk", name=f"hk{oc}", bufs=1)
                    nc.tensor.matmul(hk_ps, nh2_r, kh2_r[:, oc, :], start=True,
                                     stop=True, skip_group_check=True)
                    nc.vector.reduce_max(hkm[:, oc:oc + 1], hk_ps,
                                         axis=mybir.AxisListType.X)
                hk2_f = statp.tile([2, 1], F32)
                nc.vector.reduce_max(hk2_f, hkm, axis=mybir.AxisListType.X)
                hkt_ps = psb.tile([1, 2], F32, tag="hkt", bufs=1)
                nc.tensor.transpose(hkt_ps, hk2_f, id2_f)
                stab_sb = statp.tile([1, 1], F32)
                nc.vector.reduce_max(stab_sb, hkt_ps, axis=mybir.AxisListType.X)
                nc.sync.dma_start(out=stab_out[:, :], in_=stab_sb)

                for sc in range(2):
                    vh_ps = psb.tile([128, D_MODEL], F32, tag="vh", bufs=1)
                    for c in range(4):
                        nc.tensor.matmul(vh_ps, v_c(c)[:, ts(sc, 128)],
                                         wv_r[:, c, :], start=(c == 0),
                                         stop=(c == 3), skip_group_check=True)
                    nc.scalar.copy(vhT_sb[:, sc, :], vh_ps)
                nc.sync.dma_start(
                    out=vhT_out[:, :].rearrange("(s p) f -> p s f", p=128),
                    in_=vhT_sb)

                rsbc_ps = psb.tile([128, SLA], F32, tag="rsbc", bufs=1)
                nc.tensor.matmul(rsbc_ps, on_r, rstd_r, start=True, stop=True,
                                 skip_group_check=True)
                nc.scalar.copy(rsbc_r, rsbc_ps)

                for oc in range(4):
                    qh_ps = psb.tile([128, SLA], F32, tag="qh")
                    for c in range(4):
                        nc.tensor.matmul(qh_ps, wq_r[:, c, ts(oc, 128)],
                                         q_c(c), start=(c == 0),
                                         stop=False, skip_group_check=True)
                    nc.tensor.matmul(qh_ps, gq_r[0:1, ts(oc, 128)], mu_r,
                                     start=False, stop=True,
                                     skip_group_check=True)
                    nc.vector.tensor_mul(qh_sb[:, oc, :], qh_ps, rsbc_r)
                    nc.scalar.activation(qh_sb[:, oc, :], qh_sb[:, oc, :], IDENT,
                                         bias=cq_sb[:, oc:oc + 1], scale=1.0)
                nc.sync.dma_start(
                    out=qh_out[:, :].rearrange("(c p) f -> p c f", p=128),
                    in_=qh_sb)
    nc.compile()
    return nc


# --------------------------------------------------------------------------
# Phase B: head-sharded Performer attention + FC row-block
# --------------------------------------------------------------------------
def build_phase_b(debug=False):
    nc = bacc.Bacc("TRN2", target_bir_lowering=False, debug=False, num_devices=NC)
    khh = nc.dram_tensor("khh", [D_K, N], F32, kind="ExternalInput")
    qhh = nc.dram_tensor("qhh", [D_K, N], F32, kind="ExternalInput")
    vht = nc.dram_tensor("vht", [128, NCH * D_V], F32, kind="ExternalInput")
    rft = nc.dram_tensor("rft", [D_K, M], F32, kind="ExternalInput")
    rneg = nc.dram_tensor("rneg", [D_K, M], F32, kind="ExternalInput")
    wfc = nc.dram_tensor("wfc", [D_V, D_MODEL], F32, kind="ExternalInput")
    pairmask = nc.dram_tensor("pairmask", [C, 2 * C], F32, kind="ExternalInput")
    identm = nc.dram_tensor("identm", [128, 128], F32, kind="ExternalInput")
    onescol = nc.dram_tensor("onescol", [128, 1], F32, kind="ExternalInput")
    stabcol = nc.dram_tensor("stabcol", [128, 1], F32, kind="ExternalInput")
    c2 = nc.dram_tensor("c2", [128, 2 * NCH], F32, kind="ExternalInput")
    epsk = nc.dram_tensor("epsk", [128, NCH], F32, kind="ExternalInput")
    eps_row = nc.dram_tensor("eps_row", [1, N], F32, kind="ExternalInput")
    zeros66 = nc.dram_tensor("zeros66", [128, 3 * 66], F32, kind="ExternalInput")
    misc = nc.dram_tensor("misc", [1, 1], F32, kind="ExternalInput")  # M*eps
    out_d = nc.dram_tensor("out", [N, D_MODEL], F32, kind="ExternalOutput")

    with tile.TileContext(nc) as tc:
        with (
            tc.tile_pool(name="consts", bufs=1) as consts,
            tc.tile_pool(name="krows", bufs=1) as krows,
            tc.tile_pool(name="feat", bufs=1) as feat,
            tc.tile_pool(name="ktrp", bufs=1) as ktrp,
            tc.tile_pool(name="ktT", bufs=NCH) as ktTp,
            tc.tile_pool(name="atp", bufs=2) as atp,
            tc.tile_pool(name="ssb", bufs=NPAIR + 1) as ssbp,
            tc.tile_pool(name="post", bufs=4) as post,
            tc.tile_pool(name="outp", bufs=3) as outp,
        ):
            # ---- casting loads ----
            qhr = krows.tile([D_K, N], F32R)
            nc.gpsimd.dma_start(out=qhr, in_=qhh[:, :])
            khr = krows.tile([D_K, N], F32R)
            nc.gpsimd.dma_start(out=khr, in_=khh[:, :])
            vha = krows.tile([128, NCH, 66], F32R)
            nc.gpsimd.dma_start(
                out=vha[:, :, 0:D_V],
                in_=vht[:, :].rearrange("p (ch f) -> p ch f", ch=NCH))

            # ---- plain loads + engine conversions for small consts ----
            rft_f = consts.tile([D_K, M], F32)
            nc.sync.dma_start(out=rft_f, in_=rft[:, :])
            rft_r = consts.tile([D_K, M], F32R)
            nc.vector.tensor_copy(rft_r, rft_f)
            rneg_f = consts.tile([D_K, M], F32)
            nc.sync.dma_start(out=rneg_f, in_=rneg[:, :])
            rneg_r = consts.tile([D_K, M], F32R)
            nc.gpsimd.tensor_copy(rneg_r, rneg_f)
            wfc_f = consts.tile([D_V, D_MODEL], F32)
            nc.sync.dma_start(out=wfc_f, in_=wfc[:, :])
            wfc_r = consts.tile([D_V, D_MODEL], F32R)
            nc.vector.tensor_copy(wfc_r, wfc_f)
            id_f = consts.tile([128, 128], F32)
            nc.sync.dma_start(out=id_f, in_=identm[:, :])
            id_r = consts.tile([128, 128], F32R)
            nc.gpsimd.tensor_copy(id_r, id_f)
            onc_f = consts.tile([128, 1], F32)
            nc.sync.dma_start(out=onc_f, in_=onescol[:, :])
            onc_r = consts.tile([128, 1], F32R)
            nc.vector.tensor_copy(onc_r, onc_f)
            pmask_sb = consts.tile([C, 2 * C], F32)
            nc.sync.dma_start(out=pmask_sb, in_=pairmask[:, :])
            stab_sb = consts.tile([128, 1], F32)
            nc.sync.dma_start(out=stab_sb, in_=stabcol[:, :])
            c2_sb = consts.tile([128, 2 * NCH], F32)
            nc.sync.dma_start(out=c2_sb, in_=c2[:, :])
            epsk_sb = consts.tile([128, NCH], F32)
            nc.sync.dma_start(out=epsk_sb, in_=epsk[:, :])
            z66_sb = consts.tile([128, 3, 66], F32)
            nc.sync.dma_start(
                out=z66_sb, in_=zeros66[:, :].rearrange("p (a b) -> p a b", a=3))
            misc_sb = consts.tile([1, 1], F32)
            nc.sync.dma_start(out=misc_sb, in_=misc[:, :])

            # vha constant columns: 64 -> 1.0, 65 -> 0.0
            nc.gpsimd.tensor_copy(
                vha[:, :, D_V:D_V + 2],
                c2_sb[:, :].rearrange("p (ch f) -> p ch f", ch=NCH))

            # kh^2 rows (squares of the casting-DMA'd kh)
            kh2r = krows.tile([D_K, N], F32R)
            for s in range(NSL):
                nc.vector.tensor_mul(kh2r[:, ts(s, SL)], khr[:, ts(s, SL)],
                                     khr[:, ts(s, SL)])

            # ---- q~ features [m-major: 128 x 3 x N], interleaved with the
            # k-feature/scan pipeline at half-sequence granularity so the
            # activation engine never drains ----
            qt_feat = feat.tile([128, 3, N], F32R)
            nc.gpsimd.dma_start(out=qt_feat[11:12, 2, :], in_=eps_row[:, :])
            ktr = ktrp.tile([128, NCH, 268], F32R)
            nc.vector.tensor_copy(
                ktr[:, :, 266:267],
                epsk_sb[:, :].rearrange("p (ch f) -> p ch f", ch=NCH))

            s_tiles = [ssbp.tile([128, 3, 66], F32R, tag="ssb", name=f"ssb{i}")
                       for i in range(NPAIR + 1)]
            nc.vector.tensor_copy(s_tiles[0][:, :, :], z66_sb[:, :, :])

            ktT_tiles = {}
            at1_list = []
            at2_list = []
            zcol2 = feat.tile([128, 3, 2], F32R)
            nc.vector.tensor_copy(zcol2[:, :, 1:2], z66_sb[:, :, 0:1])
            with (
                tc.tile_pool(name="psqp", bufs=1, space="PSUM") as psqp,
                tc.tile_pool(name="qtmp2", bufs=2) as qtmp2,
                tc.tile_pool(name="pskp", bufs=2, space="PSUM") as pskp,
                tc.tile_pool(name="pstr", bufs=2, space="PSUM") as pstr,
                tc.tile_pool(name="psat", bufs=2, space="PSUM") as psat,
                tc.tile_pool(name="pssd", bufs=1, space="PSUM") as pssd,
            ):
              for half in range(2):
                for s in (2 * half, 2 * half + 1):
                    for mc in range(3):
                        mrows = 128 if mc < 2 else 10
                        qp_ps = psqp.tile([128, SL], F32, tag="qp")
                        nc.tensor.matmul(
                            qp_ps[0:mrows, :], rft_r[:, ds(mc * 128, mrows)],
                            qhr[:, ts(s, SL)], start=True, stop=True,
                            skip_group_check=True)
                        nc.scalar.activation(
                            qt_feat[0:mrows, mc, ts(s, SL)],
                            qp_ps[0:mrows, :], EXP, bias=0.0, scale=1.0)
                    sp_full = psqp.tile([128, SL], F32, tag="qp", name=f"spf{s}")
                    sp_ps = sp_full[0:1, :]
                    nc.tensor.matmul(sp_ps, onc_r, qt_feat[:, 0, ts(s, SL)],
                                     start=True, stop=False, skip_group_check=True)
                    nc.tensor.matmul(sp_ps, onc_r, qt_feat[:, 1, ts(s, SL)],
                                     start=False, stop=False, skip_group_check=True)
                    nc.tensor.matmul(sp_ps, onc_r[0:10, :],
                                     qt_feat[0:10, 2, ts(s, SL)],
                                     start=False, stop=True, skip_group_check=True)
                    sp_sb = qtmp2.tile([1, SL], F32, tag="sp_sb")
                    nc.scalar.activation(sp_sb, sp_ps, IDENT,
                                         bias=misc_sb[0:1, 0:1], scale=1.0)
                    nc.gpsimd.dma_start(out=qt_feat[10:11, 2, ts(s, SL)],
                                        in_=sp_sb)
                for ch in range(8 * half, 8 * half + 8):
                      # k features for chunk ch
                      kp_ps = pskp.tile([C, M], F32, tag="kp")
                      nc.tensor.matmul(kp_ps, khr[:, ts(ch, C)], rft_r,
                                       start=True, stop=False, skip_group_check=True)
                      nc.tensor.matmul(kp_ps, kh2r[:, ts(ch, C)], rneg_r,
                                       start=False, stop=True, skip_group_check=True)
                      with nc.allow_low_precision(reason="fp32r accum ~ fp32"):
                          nc.scalar.activation(
                              ktr[:, ch, 0:M], kp_ps, EXP,
                              bias=stab_sb[:, 0:1], scale=1.0,
                              accum_out=ktr[:, ch, 267:268])
                      # transpose the chunk's features to m-major
                      tp_ps = pstr.tile([128, 3, 128], F32R, tag="tp")
                      nc.tensor.transpose(tp_ps[:, 0, :], ktr[:, ch, 0:128], id_r)
                      nc.tensor.transpose(tp_ps[:, 1, :], ktr[:, ch, 128:256], id_r)
                      nc.tensor.transpose(tp_ps[0:12, 2, :], ktr[:, ch, 256:268], id_r)
                      ktT = ktTp.tile([128, 3, C], F32R, tag="ktT", name=f"ktT{ch}")
                      if ch % 2 == 0:
                          nc.vector.tensor_copy(ktT[:, 0:2, :], tp_ps[:, 0:2, :])
                          nc.scalar.copy(ktT[0:12, 2:3, :], tp_ps[0:12, 2:3, :])
                      else:
                          nc.scalar.copy(ktT[:, 0:2, :], tp_ps[:, 0:2, :])
                          nc.vector.tensor_copy(ktT[0:12, 2:3, :], tp_ps[0:12, 2:3, :])
                      ktT_tiles[ch] = ktT

                      if ch % 2 == 1:
                          i = ch // 2
                          a, b = 2 * i, 2 * i + 1
                          # pair attention block: keys a x queries {a, b}
                          atc_ps = psat.tile([C, 3 * C], F32, tag="atc")
                          at1_ps = atc_ps[:, 0:2 * C]
                          ka = ktT_tiles[a]
                          for mc in range(2):
                              nc.tensor.matmul(at1_ps, ka[:, mc, :],
                                               qt_feat[:, mc, ts(i, 2 * C)],
                                               start=(mc == 0), stop=False,
                                               skip_group_check=True)
                          nc.tensor.matmul(at1_ps, ka[0:12, 2, :],
                                           qt_feat[0:12, 2, ts(i, 2 * C)],
                                           start=False, stop=True,
                                           skip_group_check=True)
                          at1_r = atp.tile([C, 2 * C], F32R, tag="at1r",
                                           name=f"at1r{i}", bufs=NPAIR)
                          nc.vector.tensor_mul(at1_r, at1_ps, pmask_sb)
                          at1_list.append(at1_r)
                          # odd diag block: keys b x queries b
                          at2_ps = atc_ps[:, 2 * C:3 * C]
                          kb = ktT_tiles[b]
                          for mc in range(2):
                              nc.tensor.matmul(at2_ps, kb[:, mc, :],
                                               qt_feat[:, mc, ts(b, C)],
                                               start=(mc == 0), stop=False,
                                               skip_group_check=True)
                          nc.tensor.matmul(at2_ps, kb[0:12, 2, :],
                                           qt_feat[0:12, 2, ts(b, C)],
                                           start=False, stop=True,
                                           skip_group_check=True)
                          at2_r = atp.tile([C, C], F32R, tag="at2r",
                                           name=f"at2r{i}", bufs=NPAIR)
                          nc.vector.tensor_mul(at2_r, at2_ps, pmask_sb[:, 0:C])
                          at2_list.append(at2_r)

                          # state update for the pair
                          sd_ps = pssd.tile([128, 3, 66], F32, tag="sd")
                          if i < 2:
                              # one-time zero of rows never written by the mc2
                              # matmuls (start_tensor_calc leaves them stale);
                              # zeros persist across psum reuse (2 bufs)
                              nc.vector.memset(sd_ps[:, 2:3, :], 0.0)
                          for mc, cols in ((0, (0, 128)), (1, (128, 256)),
                                           (2, (256, 268))):
                              dst = sd_ps[:, mc, :] if mc < 2 else sd_ps[0:12, 2, :]
                              for j, ch2 in enumerate((a, b)):
                                  nc.tensor.matmul(dst, ktr[:, ch2, cols[0]:cols[1]],
                                                   vha[:, ch2, 0:66], start=(j == 0),
                                                   stop=(j == 1),
                                                   skip_group_check=True)
                          nc.vector.tensor_add(s_tiles[i + 1][:, :, 0:66],
                                               s_tiles[i][:, :, 0:66],
                                               sd_ps[:, :, :])

            # z column = final state's column 64 (already column-major)
            zfin = s_tiles[NPAIR]
            nc.vector.tensor_copy(zcol2[:, 0:2, 0:1], zfin[:, 0:2, 64:65])
            nc.scalar.copy(zcol2[0:12, 2:3, 0:1], zfin[0:12, 2:3, 64:65])

            # ---- per-chunk output, normalize, FC ----
            with (
                tc.tile_pool(name="pso", bufs=3, space="PSUM") as pso,
                tc.tile_pool(name="psfc", bufs=2, space="PSUM") as psfc,
                tc.tile_pool(name="pstr2", bufs=2, space="PSUM") as pstr2,
            ):
                for ch in range(NCH):
                    i = ch // 2
                    o_ps = pso.tile([C, 68], F32, tag="o")
                    for mc in range(2):
                        nc.tensor.matmul(o_ps[:, 0:66], qt_feat[:, mc, ts(ch, C)],
                                         s_tiles[i][:, mc, 0:66],
                                         start=(mc == 0), stop=False,
                                         skip_group_check=True)
                    nc.tensor.matmul(o_ps[:, 0:66], qt_feat[0:12, 2, ts(ch, C)],
                                     s_tiles[i][0:12, 2, 0:66],
                                     start=False, stop=False, skip_group_check=True)
                    if ch % 2 == 0:
                        nc.tensor.matmul(
                            o_ps[:, 0:66], at1_list[i][:, 0:C],
                            vha[:, ch, :], start=False, stop=True,
                            skip_group_check=True)
                    else:
                        nc.tensor.matmul(
                            o_ps[:, 0:66], at1_list[i][:, C:2 * C],
                            vha[:, ch - 1, :], start=False, stop=False,
                            skip_group_check=True)
                        nc.tensor.matmul(
                            o_ps[:, 0:66], at2_list[i],
                            vha[:, ch, :], start=False, stop=True,
                            skip_group_check=True)
                    # d = q~ . z in columns 66:68 (col 67 is a zero pad)
                    for mc in range(2):
                        nc.tensor.matmul(o_ps[:, 66:68], qt_feat[:, mc, ts(ch, C)],
                                         zcol2[:, mc, :], start=(mc == 0),
                                         stop=False, skip_group_check=True)
                    nc.tensor.matmul(o_ps[:, 66:68], qt_feat[0:12, 2, ts(ch, C)],
                                     zcol2[0:12, 2, :], start=False, stop=True,
                                     skip_group_check=True)
                    dcols = post.tile([C, 2, 2], F32, tag="dcols")
                    nc.vector.tensor_copy(
                        dcols, o_ps[:, 64:68].rearrange("p (a b) -> p a b", a=2))
                    dd = post.tile([C, 1], F32, tag="dd")
                    nc.gpsimd.tensor_mul(dd, dcols[:, 0, 0:1], dcols[:, 1, 0:1])
                    rec = post.tile([C, 1], F32, tag="rec")
                    nc.vector.reciprocal(rec, dd)
                    attn_r = post.tile([C, D_V], F32R, tag="attn")
                    nc.vector.tensor_scalar_mul(attn_r, o_ps[:, 0:D_V], rec)
                    tr_ps = pstr2.tile([D_V, C], F32R, tag="tr")
                    nc.tensor.transpose(tr_ps, attn_r, id_r)
                    attnT_r = post.tile([D_V, C], F32R, tag="attnT")
                    nc.scalar.copy(attnT_r, tr_ps)
                    fc_ps = psfc.tile([C, D_MODEL], F32, tag="fc")
                    nc.tensor.matmul(fc_ps, attnT_r, wfc_r, start=True, stop=True,
                                     skip_group_check=True)
                    o_sb = outp.tile([C, D_MODEL], F32, tag="osb")
                    if ch % 2 == 0:
                        nc.vector.tensor_copy(o_sb, fc_ps)
                    else:
                        nc.scalar.copy(o_sb, fc_ps)
                    nc.sync.dma_start(out=out_d[ts(ch, C), :], in_=o_sb)
    nc.compile()
    return nc


# --------------------------------------------------------------------------
# Host orchestration
# --------------------------------------------------------------------------
_CACHE = {}


def _get_programs():
    if "a" not in _CACHE:
        _CACHE["a"] = build_phase_a()
        _CACHE["b"] = build_phase_b()
    return _CACHE["a"], _CACHE["b"]


def _prep_a_maps(q, k, v, Wq, Wk, Wv, gamma, beta):
    qT = np.ascontiguousarray(q[0].T)
    kT = np.ascontiguousarray(k[0].T)
    vT = np.ascontiguousarray(v[0].T)
    Wqe = np.ascontiguousarray((gamma[:, None] * Wq) * SCALE)
    Wke = np.ascontiguousarray(Wk * SCALE)
    cq_all = (beta @ Wq) * SCALE                       # [512]
    cq = np.ascontiguousarray(cq_all.reshape(4, 128).T)
    wmean = np.full((128, 1), 1.0 / D_MODEL, np.float32)
    nh2 = np.zeros((128, 2), np.float32)
    nh2[0:64, 0] = -0.5
    nh2[64:128, 1] = -0.5
    ones_r = np.ones((1, 128), np.float32)
    gqneg = np.ascontiguousarray(-Wqe.sum(axis=0)[None, :])
    ident2 = np.eye(2, dtype=np.float32)
    misc = np.full((1, 1), LN_EPS, np.float32)
    in_a = []
    for j in range(NC):
        sl = slice(j * SLA, (j + 1) * SLA)
        xs = np.concatenate([qT[:, sl], kT[:, sl], vT[:, sl]], axis=1)
        in_a.append({
            "xs": np.ascontiguousarray(xs),
            "Wqe": Wqe, "Wke": Wke, "Wv": np.ascontiguousarray(Wv),
            "cq": cq, "wmean": wmean, "nh2": nh2, "ones_r": ones_r,
            "gqneg": gqneg, "ident2": ident2, "misc": misc,
        })
    return in_a


def _prep_b_maps(W_fc, rf, res_a):
    kh_full = np.concatenate([r["kh"] for r in res_a], axis=1)   # [512, N]
    qh_full = np.concatenate([r["qh"] for r in res_a], axis=1)   # [512, N]
    vh_full = np.concatenate([r["vhT"] for r in res_a], axis=0)  # [N, 512]
    k_stab = np.float32(max(r["stab"][0, 0] for r in res_a))

    rftT = np.ascontiguousarray(rf.T)                  # [64, 266]
    rneg = np.full((D_K, M), -0.5, np.float32)
    tri = np.triu(np.ones((C, C), np.float32))
    pairmask = np.ascontiguousarray(
        np.concatenate([tri, np.ones((C, C), np.float32)], axis=1))
    identm = np.eye(128, dtype=np.float32)
    onescol = np.ones((128, 1), np.float32)
    stabcol = np.full((128, 1), -k_stab, np.float32)
    c2 = np.zeros((128, 2 * NCH), np.float32)
    c2[:, 0::2] = 1.0                                  # vha col 64 = 1, col 65 = 0
    epsk = np.full((128, NCH), KERNEL_EPS, np.float32)
    eps_row = np.full((1, N), KERNEL_EPS, np.float32)
    zeros66 = np.zeros((128, 3 * 66), np.float32)
    misc = np.full((1, 1), M * KERNEL_EPS, np.float32)

    in_b = []
    for h in range(NC):
        rows = slice(h * D_K, (h + 1) * D_K)
        vh_h = vh_full[:, h * D_V:(h + 1) * D_V]       # [N, 64]
        vht = np.ascontiguousarray(
            vh_h.reshape(NCH, 128, D_V).transpose(1, 0, 2).reshape(128, NCH * D_V))
        in_b.append({
            "khh": np.ascontiguousarray(kh_full[rows]),
            "qhh": np.ascontiguousarray(qh_full[rows]),
            "vht": vht,
            "rft": rftT, "rneg": rneg,
            "wfc": np.ascontiguousarray(W_fc[rows, :] * float(M)),
            "pairmask": pairmask, "identm": identm, "onescol": onescol,
            "stabcol": stabcol, "c2": c2, "epsk": epsk, "eps_row": eps_row,
            "zeros66": zeros66, "misc": misc,
        })
    return in_b


def _cast_all(*arrs):
    return [np.asarray(a, np.float32) for a in arrs]


def kernel(q, k, v, Wq, Wk, Wv, W_fc, b_fc, gamma, beta, rf):
    q, k, v, Wq, Wk, Wv, W_fc, b_fc, gamma, beta, rf = _cast_all(
        q, k, v, Wq, Wk, Wv, W_fc, b_fc, gamma, beta, rf)

    nc_a, nc_b = _get_programs()
    cores = list(range(NC))

    in_a = _prep_a_maps(q, k, v, Wq, Wk, Wv, gamma, beta)
    res_a = run_bass_kernel_spmd(nc_a, in_a, core_ids=cores)

    in_b = _prep_b_maps(W_fc, rf, res_a.results)
    res_b = run_bass_kernel_spmd(nc_b, in_b, core_ids=cores)

    out = np.zeros((N, D_MODEL), np.float32)
    for r in res_b.results:
        out += r["out"]
    out += b_fc[None, :]
    out += q[0]
    return out[None].astype(np.float32)


def trace_args(inputs):
    """For test.py: returns [(phase, nc, in_maps), ...] re-runnable with trace."""
    q, k, v, Wq, Wk, Wv, W_fc, b_fc, gamma, beta, rf = _cast_all(
        inputs["q"], inputs["k"], inputs["v"], inputs["Wq"], inputs["Wk"],
        inputs["Wv"], inputs["W_fc"], inputs["b_fc"], inputs["gamma"],
        inputs["beta"], inputs["rf"])
    nc_a, nc_b = _get_programs()
    in_a = _prep_a_maps(q, k, v, Wq, Wk, Wv, gamma, beta)
    res_a = run_bass_kernel_spmd(nc_a, in_a, core_ids=list(range(NC)))
    in_b = _prep_b_maps(W_fc, rf, res_a.results)
    return [("a", nc_a, in_a), ("b", nc_b, in_b)]


if __name__ == "__main__":
    rng = np.random.default_rng(0)
    inputs = {
        "q": rng.standard_normal((1, N, D_MODEL)).astype(np.float32),
        "k": rng.standard_normal((1, N, D_MODEL)).astype(np.float32),
        "v": rng.standard_normal((1, N, D_MODEL)).astype(np.float32),
        "Wq": (rng.standard_normal((D_MODEL, 512)) * 0.04).astype(np.float32),
        "Wk": (rng.standard_normal((D_MODEL, 512)) * 0.04).astype(np.float32),
        "Wv": (rng.standard_normal((D_MODEL, 512)) * 0.04).astype(np.float32),
        "W_fc": (rng.standard_normal((512, D_MODEL)) * 0.04).astype(np.float32),
        "b_fc": np.zeros(D_MODEL, np.float32),
        "gamma": np.ones(D_MODEL, np.float32),
        "beta": np.zeros(D_MODEL, np.float32),
        "rf": rng.standard_normal((M, D_K)).astype(np.float32),
    }
    out = kernel(**inputs)
    print("kernel output", out.shape, out.dtype)



# revision 22
# speedup vs baseline: 1.0823x; 1.0823x over previous
"""Performer attention (causal, kernelized) — Trainium2 Bass kernel, v4 (bf16).

Two launches on 8 cores:

  A) seq-sharded prep: core j owns 256 sequence positions and computes, for
     ALL 8 heads at once: kh (scaled k-projection), the LayerNorm-folded and
     scaled/biased q-projection qh, the v-projection in seq-major layout,
     and the local stabilizer max(h_k).

  B) head-sharded attention: core h owns head h end-to-end: Performer
     feature maps, the causal chunked prefix scan (per-chunk states, C=128),
     output normalization and its row-block of the FC (W_fc row-sharded;
     host sums partials and adds bias + residual).

All matmuls/operands are bfloat16 (cost model: 1.0 PE cycles/row at ANY free
size, vs fp32r's 4.0 below free=256; DMA bytes halved; DVE 2-4x modes).
PSUM accumulation stays fp32.  Algebra identical to v3:
  - q LayerNorm folded: Wq_eff = diag(gamma) Wq * scale, bias cq = beta@Wq*scale,
    applied to (q - mu) * rstd with rstd = exp(-0.5 ln(var + eps)).
  - exp(h_q + (proj_q - h_q)) == exp(proj_q): q-side stabilizer cancels.
  - k feature: exp(proj_k + h_k - k_stab) via the augmented contraction
    [kh; kh^2] . [rf^T; -0.5] plus a constant bias of -k_stab in the exp.
  - +KERNEL_EPS becomes extra features: q~ rows 266/267 = (sum_m exp_q + m*eps,
    eps); k~ cols 266/267 = (eps, sum_m exp_k); the global 1/sqrt(m) cancels
    except 1/c^2 folded into W_fc.
  - causal prefix scan chunked at C=128 with per-chunk states; diagonal
    128x128 score blocks handle intra-chunk causality via a triangular mask.
  - d = q~ . z (z = column sums of k~) rides in o column 66.
  - the reference's |d|<=1e-6 guard is dead for any realistic data and is
    omitted.
"""

import sys
for _p in ("/opt/trn_rl_repo", "/root/.axon_site/_ro/trn_rl_repo"):
    if _p not in sys.path:
        sys.path.append(_p)

import numpy as np
import ml_dtypes

import concourse.bass as bass
from concourse import bacc
import concourse.mybir as mybir
import concourse.tile as tile
from concourse.bass import ts, ds
from concourse.bass_utils import run_bass_kernel_spmd

F32 = mybir.dt.float32
BF16 = mybir.dt.bfloat16
NPBF = ml_dtypes.bfloat16
NC = 8
N = 2048
D_MODEL = 512
D_K = 64
D_V = 64
M = 266
C = 128
NCH = N // C            # 16 chunks
SLA = N // NC           # 256 seq positions per phase-A core
NSL = 4                 # 512-wide slices of the full sequence
SL = 512
KERNEL_EPS = 1e-4
LN_EPS = 1e-6
SCALE = float(D_MODEL) ** (-0.25)
EXP = mybir.ActivationFunctionType.Exp
LN_F = mybir.ActivationFunctionType.Ln
IDENT = mybir.ActivationFunctionType.Identity


# --------------------------------------------------------------------------
# Phase A: seq-sharded projections + local stabilizer
# --------------------------------------------------------------------------
def build_phase_a():
    nc = bacc.Bacc("TRN2", target_bir_lowering=False, debug=False, num_devices=NC)
    xs = nc.dram_tensor("xs", [D_MODEL, 3 * SLA], BF16, kind="ExternalInput")
    Wqe = nc.dram_tensor("Wqe", [D_MODEL, D_MODEL], BF16, kind="ExternalInput")
    Wke = nc.dram_tensor("Wke", [D_MODEL, D_MODEL], BF16, kind="ExternalInput")
    Wv = nc.dram_tensor("Wv", [D_MODEL, D_MODEL], BF16, kind="ExternalInput")
    # bf16 consts packed: [128, 4]: col0 wmean, col1 ones, col2 gq rows? no-
    # gq is [1, 512] -> separate. col2 neghalf.
    cb = nc.dram_tensor("cb", [128, 4], BF16, kind="ExternalInput")
    gqneg = nc.dram_tensor("gqneg", [1, D_MODEL + 128], BF16, kind="ExternalInput")
    cq = nc.dram_tensor("cq", [128, 4], F32, kind="ExternalInput")
    misc = nc.dram_tensor("misc", [1, 1], F32, kind="ExternalInput")  # LN_EPS
    kh_out = nc.dram_tensor("kh", [D_MODEL, SLA], BF16, kind="ExternalOutput")
    qh_out = nc.dram_tensor("qh", [D_MODEL, SLA], BF16, kind="ExternalOutput")
    vhT_out = nc.dram_tensor("vhT", [SLA, D_MODEL], BF16, kind="ExternalOutput")
    stab_out = nc.dram_tensor("stab", [2, 1], F32, kind="ExternalOutput")

    with tile.TileContext(nc) as tc:
        with (
            tc.tile_pool(name="wts", bufs=1) as wts,
            tc.tile_pool(name="xin", bufs=1) as xin,
            tc.tile_pool(name="work", bufs=1) as work,
            tc.tile_pool(name="stat", bufs=1) as statp,
            tc.tile_pool(name="outs", bufs=1) as outs,
        ):
            # ---- loads; order: consts, x, Wk, Wq, Wv so compute starts early
            cb_sb = wts.tile([128, 4], BF16)
            nc.scalar.dma_start(out=cb_sb, in_=cb[:, :])
            cq_sb = wts.tile([128, 4], F32)
            nc.scalar.dma_start(out=cq_sb, in_=cq[:, :])
            misc_sb = wts.tile([1, 1], F32)
            nc.scalar.dma_start(out=misc_sb, in_=misc[:, :])
            gq_sb = wts.tile([1, D_MODEL + 128], BF16)
            nc.scalar.dma_start(out=gq_sb, in_=gqneg[:, :])
            q_r = xin.tile([128, 4, SLA], BF16)
            nc.sync.dma_start(out=q_r, in_=xs[:, 0:SLA].rearrange("(c p) f -> p c f", p=128))
            k_r = xin.tile([128, 4, SLA], BF16)
            nc.sync.dma_start(out=k_r, in_=xs[:, SLA:2 * SLA].rearrange("(c p) f -> p c f", p=128))
            v_r = xin.tile([128, 4, SLA], BF16)
            nc.sync.dma_start(out=v_r, in_=xs[:, 2 * SLA:3 * SLA].rearrange("(c p) f -> p c f", p=128))
            wk_r = wts.tile([128, 4, D_MODEL], BF16)
            nc.sync.dma_start(out=wk_r, in_=Wke[:, :].rearrange("(c p) f -> p c f", p=128))
            wq_r = wts.tile([128, 4, D_MODEL], BF16)
            nc.sync.dma_start(out=wq_r, in_=Wqe[:, :].rearrange("(c p) f -> p c f", p=128))
            wv_r = wts.tile([128, 4, D_MODEL], BF16)
            nc.gpsimd.dma_start(out=wv_r, in_=Wv[:, :].rearrange("(c p) f -> p c f", p=128))

            wm_r = cb_sb[:, 0:1]      # 1/512
            on_r = cb_sb[:, 1:2]      # ones col [128,1]
            nh_r = cb_sb[:, 2:4]      # -0.5 split into per-head halves

            def q_c(c):
                return q_r[:, c, :]

            def k_c(c):
                return k_r[:, c, :]

            def v_c(c):
                return v_r[:, c, :]

            # ---- LayerNorm stats on q (over d_model, per position) ----
            mu_r = statp.tile([1, SLA], BF16)
            with tc.tile_pool(name="pss", bufs=1, space="PSUM") as pss:
                mu_ps = pss.tile([1, SLA], F32, tag="mu")
                for c in range(4):
                    nc.tensor.matmul(mu_ps, wm_r, q_c(c), start=(c == 0),
                                     stop=(c == 3), skip_group_check=True)
                qsq_r = work.tile([128, 4, SLA], BF16)
                nc.vector.tensor_mul(qsq_r, q_r, q_r)
                msq_ps = pss.tile([1, SLA], F32, tag="msq")
                for c in range(4):
                    nc.tensor.matmul(msq_ps, wm_r, qsq_r[:, c, :], start=(c == 0),
                                     stop=(c == 3), skip_group_check=True)
                mu_f = statp.tile([1, SLA], F32)
                nc.vector.tensor_copy(mu_f, mu_ps)
                nc.vector.tensor_copy(mu_r, mu_ps)
                var_sb = statp.tile([1, SLA], F32)
                nc.vector.tensor_mul(var_sb, mu_f, mu_f)
                nc.vector.tensor_sub(var_sb, msq_ps, var_sb)
                rstd_r = statp.tile([1, SLA], BF16)
                nc.scalar.activation(rstd_r, var_sb, LN_F,
                                     bias=misc_sb[0:1, 0:1], scale=1.0)
                nc.scalar.activation(rstd_r, rstd_r, EXP, bias=0.0, scale=-0.5)

            # ---- projections: kh first (only needs Wk), then qh, then vh ----
            kh_sb = outs.tile([128, 4, SLA], BF16)
            kh2_r = work.tile([128, 4, SLA], BF16)
            qh_sb = outs.tile([128, 4, SLA], BF16)
            vhT_sb = outs.tile([128, 2, D_MODEL], BF16)
            with tc.tile_pool(name="psb", bufs=2, space="PSUM") as psb:
                for oc in range(4):
                    kh_ps = psb.tile([128, SLA], F32, tag="kh")
                    for c in range(4):
                        nc.tensor.matmul(kh_ps, wk_r[:, c, ts(oc, 128)],
                                         k_c(c), start=(c == 0),
                                         stop=(c == 3), skip_group_check=True)
                    nc.scalar.copy(kh_sb[:, oc, :], kh_ps)
                    nc.vector.tensor_mul(kh2_r[:, oc, :], kh_sb[:, oc, :],
                                         kh_sb[:, oc, :])
                nc.sync.dma_start(
                    out=kh_out[:, :].rearrange("(c p) f -> p c f", p=128),
                    in_=kh_sb)

                # local stabilizer max over heads/positions of -0.5||kh_h||^2;
                # each 128-partition oc chunk holds 2 heads (64 dims each)
                hkm = statp.tile([2, 4], F32)
                for oc in range(4):
                    hk_ps = psb.tile([2, SLA], F32, tag="hk", bufs=1,
                                     name=f"hk{oc}")
                    nc.tensor.matmul(hk_ps, nh_r, kh2_r[:, oc, :],
                                     start=True, stop=True,
                                     skip_group_check=True)
                    nc.vector.reduce_max(hkm[:, oc:oc + 1], hk_ps,
                                         axis=mybir.AxisListType.X)
                stab_sb = statp.tile([2, 1], F32)
                nc.vector.reduce_max(stab_sb, hkm, axis=mybir.AxisListType.X)
                nc.scalar.dma_start(out=stab_out[:, :], in_=stab_sb)

                # rstd broadcast to 128 partitions
                rsbc_sb = work.tile([128, SLA], BF16)
                rsbc_ps = psb.tile([128, SLA], F32, tag="rsbc", bufs=1)
                nc.tensor.matmul(rsbc_ps, gq_sb[0:1, D_MODEL:D_MODEL + 128], rstd_r, start=True, stop=True,
                                 skip_group_check=True)
                nc.scalar.copy(rsbc_sb, rsbc_ps)

                for oc in range(4):
                    qh_ps = psb.tile([128, SLA], F32, tag="qh")
                    for c in range(4):
                        nc.tensor.matmul(qh_ps, wq_r[:, c, ts(oc, 128)],
                                         q_c(c), start=(c == 0),
                                         stop=False, skip_group_check=True)
                    nc.tensor.matmul(qh_ps, gq_sb[0:1, ts(oc, 128)], mu_r,
                                     start=False, stop=True,
                                     skip_group_check=True)
                    nc.vector.tensor_mul(qh_sb[:, oc, :], qh_ps, rsbc_sb)
                    nc.scalar.activation(qh_sb[:, oc, :], qh_sb[:, oc, :], IDENT,
                                         bias=cq_sb[:, oc:oc + 1], scale=1.0)
                nc.sync.dma_start(
                    out=qh_out[:, :].rearrange("(c p) f -> p c f", p=128),
                    in_=qh_sb)

                for sc in range(2):
                    vh_ps = psb.tile([128, D_MODEL], F32, tag="vh")
                    for c in range(4):
                        nc.tensor.matmul(vh_ps, v_c(c)[:, ts(sc, 128)],
                                         wv_r[:, c, :], start=(c == 0),
                                         stop=(c == 3), skip_group_check=True)
                    if sc == 0:
                        nc.scalar.copy(vhT_sb[:, sc, :], vh_ps)
                    else:
                        nc.vector.tensor_copy(vhT_sb[:, sc, :], vh_ps)
                nc.sync.dma_start(
                    out=vhT_out[:, :].rearrange("(s p) f -> p s f", p=128),
                    in_=vhT_sb)
    nc.compile()
    return nc


# --------------------------------------------------------------------------
# Phase B: head-sharded Performer attention + FC row-block
# --------------------------------------------------------------------------
def build_phase_b(debug=False):
    nc = bacc.Bacc("TRN2", target_bir_lowering=False, debug=False, num_devices=NC)
    khh = nc.dram_tensor("khh", [D_K, N], BF16, kind="ExternalInput")
    qhh = nc.dram_tensor("qhh", [D_K, N], BF16, kind="ExternalInput")
    vht = nc.dram_tensor("vht", [128, NCH * D_V], BF16, kind="ExternalInput")
    # bf16 const blob [128, 128+128+266+266+512+1]: idm|tri|rft|rneg|wfc|onc
    NBLOB = 128 + 128 + M + M + D_MODEL + 1
    blob = nc.dram_tensor("blob", [128, NBLOB], BF16, kind="ExternalInput")
    stabc = nc.dram_tensor("stabc", [128, 129], F32, kind="ExternalInput")
    eps_row = nc.dram_tensor("eps_row", [1, N], BF16, kind="ExternalInput")
    misc = nc.dram_tensor("misc", [1, 1], F32, kind="ExternalInput")  # M*eps
    out_d = nc.dram_tensor("out", [N, D_MODEL], BF16, kind="ExternalOutput")
    if debug:
        dbg_at = nc.dram_tensor("dbg_at", [C, 3 * C], BF16, kind="ExternalOutput")
        dbg_qt = nc.dram_tensor("dbg_qt", [128, 3 * 256], BF16, kind="ExternalOutput")
        dbg_s = nc.dram_tensor("dbg_s", [128, 3 * 66], BF16, kind="ExternalOutput")
        dbg_z = nc.dram_tensor("dbg_z", [128, 3], BF16, kind="ExternalOutput")
        dbg_kt = nc.dram_tensor("dbg_kt", [128, 3 * 290], BF16, kind="ExternalOutput")
        dbg_o = nc.dram_tensor("dbg_o", [C, 3 * 67], F32, kind="ExternalOutput")
        dbg_rec = nc.dram_tensor("dbg_rec", [C, 3], F32, kind="ExternalOutput")
        dbg_att = nc.dram_tensor("dbg_att", [C, 3 * D_V], BF16, kind="ExternalOutput")
        dbg_attT = nc.dram_tensor("dbg_attT", [D_V, 3 * C], BF16, kind="ExternalOutput")
        dbg_fc = nc.dram_tensor("dbg_fc", [C, 3 * D_MODEL], F32, kind="ExternalOutput")

    with tile.TileContext(nc) as tc:
        with (
            tc.tile_pool(name="consts", bufs=1) as consts,
            tc.tile_pool(name="krows", bufs=1) as krows,
            tc.tile_pool(name="feat", bufs=1) as feat,
            tc.tile_pool(name="ktrp", bufs=1) as ktrp,
            tc.tile_pool(name="ktT", bufs=NCH) as ktTp,
            tc.tile_pool(name="atp", bufs=NCH) as atp,
            tc.tile_pool(name="ssb", bufs=NCH + 1) as ssbp,
            tc.tile_pool(name="post", bufs=4) as post,
            tc.tile_pool(name="outp", bufs=3) as outp,
        ):
            # ---- loads ----
            blob_sb = consts.tile([128, NBLOB], BF16)
            nc.scalar.dma_start(out=blob_sb, in_=blob[:, :])
            stab_full = consts.tile([128, 129], F32)
            nc.scalar.dma_start(out=stab_full, in_=stabc[:, :])
            stab_sb = stab_full[:, 0:1]
            idf_r = stab_full[:, 1:129]
            misc_sb = consts.tile([1, 1], F32)
            nc.scalar.dma_start(out=misc_sb, in_=misc[:, :])
            qhr = krows.tile([D_K, N], BF16)
            nc.sync.dma_start(out=qhr, in_=qhh[:, :])
            khr = krows.tile([D_K, N], BF16)
            nc.sync.dma_start(out=khr, in_=khh[:, :])
            vha = krows.tile([128, NCH, 66], BF16)
            nc.gpsimd.dma_start(
                out=vha[:, :, 0:D_V],
                in_=vht[:, :].rearrange("p (ch f) -> p ch f", ch=NCH))

            id_r = blob_sb[:, 0:128]
            tri_r = blob_sb[:, 128:256]
            rft_r = blob_sb[0:D_K, 256:256 + M]
            rneg_r = blob_sb[0:D_K, 256 + M:256 + 2 * M]
            wfc_r = blob_sb[0:D_V, 256 + 2 * M:256 + 2 * M + D_MODEL]
            onc_r = blob_sb[:, NBLOB - 1:NBLOB]

            # vha constant columns: 64 -> 1.0, 65 -> 0.0
            nc.gpsimd.memset(vha[:, :, D_V:D_V + 1], 1.0)
            nc.gpsimd.memset(vha[:, :, D_V + 1:D_V + 2], 0.0)

            # kh^2 rows
            kh2r = krows.tile([D_K, N], BF16)
            nc.vector.tensor_mul(kh2r, khr, khr)

            # q~ features m-major [128, 3, N]; mc2 rows: 0..9 = features
            # 256..265, 10..31 unused, 32 = sum-feature, 33 = eps
            qt_feat = feat.tile([128, 3, N], BF16)
            nc.gpsimd.memset(qt_feat[0:32, 2, :], 0.0)
            nc.scalar.dma_start(out=qt_feat[33:34, 2, :], in_=eps_row[:, :])
            # k~ seq-major [C, ch, 290]; cols 0..265 = exp features,
            # 266..287 = zeros, 288 = eps, 289 = sum
            ktr = ktrp.tile([128, NCH, 290], BF16)
            nc.gpsimd.memset(ktr[:, :, 266:288], 0.0)
            nc.gpsimd.memset(ktr[:, :, 288:289], KERNEL_EPS)

            s_tiles = [ssbp.tile([128, 3, 66], BF16, tag="ssb", name=f"ssb{i}")
                       for i in range(NCH + 1)]

            ktT_tiles = {}
            at_list = []
            with (
                tc.tile_pool(name="psq", bufs=2, space="PSUM") as psq,
                tc.tile_pool(name="pskp", bufs=2, space="PSUM") as pskp,
                tc.tile_pool(name="pstr", bufs=1, space="PSUM") as pstr,
                tc.tile_pool(name="psat", bufs=1, space="PSUM") as psat,
                tc.tile_pool(name="pssd", bufs=1, space="PSUM") as pssd,
            ):
              for s in range(NSL):
                    # ---- q features for slice s ----
                    for mc in range(2):
                        qp_ps = psq.tile([128, SL], F32, tag="qp")
                        nc.tensor.matmul(
                            qp_ps, rft_r[:, ts(mc, 128)],
                            qhr[:, ts(s, SL)], start=True, stop=True,
                            skip_group_check=True)
                        nc.scalar.activation(
                            qt_feat[:, mc, ts(s, SL)],
                            qp_ps, EXP, bias=0.0, scale=1.0)
                    qp2_ps = psq.tile([33, SL], F32, tag="qp2", bufs=1)
                    nc.tensor.matmul(
                        qp2_ps[0:10, :], rft_r[:, ds(256, 10)],
                        qhr[:, ts(s, SL)], start=True, stop=True,
                        skip_group_check=True)
                    nc.scalar.activation(
                        qt_feat[0:10, 2, ts(s, SL)],
                        qp2_ps[0:10, :], EXP, bias=0.0, scale=1.0)
                    # sum-feature row: ones over the 266 exp rows (+ M*eps)
                    nc.tensor.matmul(qp2_ps[32:33, :], onc_r,
                                     qt_feat[:, 0, ts(s, SL)],
                                     start=True, stop=False, skip_group_check=True)
                    nc.tensor.matmul(qp2_ps[32:33, :], onc_r,
                                     qt_feat[:, 1, ts(s, SL)],
                                     start=False, stop=False, skip_group_check=True)
                    nc.tensor.matmul(qp2_ps[32:33, :], onc_r[0:10, :],
                                     qt_feat[0:10, 2, ts(s, SL)],
                                     start=False, stop=True, skip_group_check=True)
                    nc.vector.tensor_scalar_add(
                        qt_feat[32:33, 2, ts(s, SL)], qp2_ps[32:33, :],
                        misc_sb[0:1, 0:1])

                    # ---- k features + scan for the 4 chunks of slice s ----
                    for ch in range(4 * s, 4 * s + 4):
                        kp_ps = pskp.tile([C, M], F32, tag="kp")
                        nc.tensor.matmul(kp_ps, khr[:, ts(ch, C)], rft_r,
                                         start=True, stop=False,
                                         skip_group_check=True)
                        nc.tensor.matmul(kp_ps, kh2r[:, ts(ch, C)], rneg_r,
                                         start=False, stop=True,
                                         skip_group_check=True)
                        with nc.allow_low_precision(reason="bf16 features"):
                            nc.scalar.activation(
                                ktr[:, ch, 0:M], kp_ps, EXP,
                                bias=stab_sb[:, 0:1], scale=1.0)
                        with nc.allow_low_precision(reason="bf16 feature sum"):
                            nc.vector.reduce_sum(ktr[:, ch, 289:290],
                                                 ktr[:, ch, 0:M],
                                                 axis=mybir.AxisListType.X)
                        # transpose k~ chunk to m-major
                        tp_ps = pstr.tile([128, 3, 128], BF16, tag="tp")
                        nc.tensor.transpose(tp_ps[:, 0, :], ktr[:, ch, 0:128], id_r)
                        nc.tensor.transpose(tp_ps[:, 1, :], ktr[:, ch, 128:256], id_r)
                        nc.tensor.transpose(tp_ps[0:34, 2, :], ktr[:, ch, 256:290], id_r)
                        ktT = ktTp.tile([128, 3, C], BF16, tag="ktT", name=f"ktT{ch}")
                        nc.scalar.copy(ktT[:, 0, :], tp_ps[:, 0, :])
                        nc.vector.tensor_copy(ktT[:, 1, :], tp_ps[:, 1, :])
                        (nc.scalar.copy if ch % 2 else nc.vector.tensor_copy)(ktT[0:34, 2, :], tp_ps[0:34, 2, :])
                        ktT_tiles[ch] = ktT

                        # diagonal score block (keys ch x queries ch), masked
                        at_ps = psat.tile([C, C], F32, tag="at")
                        for mc in range(2):
                            nc.tensor.matmul(at_ps, ktT[:, mc, :],
                                             qt_feat[:, mc, ts(ch, C)],
                                             start=(mc == 0), stop=False,
                                             skip_group_check=True)
                        nc.tensor.matmul(at_ps, ktT[0:34, 2, :],
                                         qt_feat[0:34, 2, ts(ch, C)],
                                         start=False, stop=True,
                                         skip_group_check=True)
                        at_r = atp.tile([C, C], BF16, tag="at_r", name=f"atr{ch}")
                        nc.vector.tensor_mul(at_r, at_ps, tri_r)
                        at_list.append(at_r)

                        # state update for this chunk
                        sd_ps = pssd.tile([128, 3, 66], F32, tag="sd")
                        if ch == 0:
                            # one-time zero of mc2 rows the matmuls never
                            # write; persists across psum reuse (bufs=1)
                            nc.vector.memset(sd_ps[:, 2, :], 0.0)
                        for mc, cols in ((0, (0, 128)), (1, (128, 256)),
                                         (2, (256, 290))):
                            dst = sd_ps[:, mc, :] if mc < 2 else sd_ps[0:34, 2, :]
                            nc.tensor.matmul(dst, ktr[:, ch, cols[0]:cols[1]],
                                             vha[:, ch, 0:66], start=True,
                                             stop=True, skip_group_check=True)
                        if ch == 0:
                            nc.vector.tensor_copy(s_tiles[1][:, :, :], sd_ps)
                        else:
                            nc.vector.tensor_add(s_tiles[ch + 1][:, :, :],
                                                 s_tiles[ch][:, :, :], sd_ps)

            # z column = final state's column 64
            zcol = feat.tile([128, 3, 1], BF16)
            nc.vector.tensor_copy(zcol, s_tiles[NCH][:, :, 64:65])
            if debug:
                for i in range(3):
                    nc.sync.dma_start(out=dbg_at[:, ts(i, C)], in_=at_list[i])
                nc.sync.dma_start(
                    out=dbg_qt[:, :].rearrange("p (a b) -> p a b", a=3),
                    in_=qt_feat[:, :, 0:256])
                nc.sync.dma_start(
                    out=dbg_s[:, :].rearrange("p (a b) -> p a b", a=3),
                    in_=s_tiles[2][:, :, :])
                nc.sync.dma_start(out=dbg_z[:, :], in_=zcol[:, :, 0])
                nc.sync.dma_start(
                    out=dbg_kt[:, :].rearrange("p (a b) -> p a b", a=3),
                    in_=ktr[:, 0:3, :])

            # ---- per-chunk output, normalize, FC ----
            with (
                tc.tile_pool(name="pso", bufs=3, space="PSUM") as pso,
                tc.tile_pool(name="psfc", bufs=2, space="PSUM") as psfc,
                tc.tile_pool(name="pstr2", bufs=2, space="PSUM") as pstr2,
            ):
                for ch in range(NCH):
                    o_ps = pso.tile([C, 67], F32, tag="o")
                    if ch > 0:
                        for mc in range(2):
                            nc.tensor.matmul(o_ps[:, 0:66],
                                             qt_feat[:, mc, ts(ch, C)],
                                             s_tiles[ch][:, mc, 0:66],
                                             start=(mc == 0), stop=False,
                                             skip_group_check=True)
                        nc.tensor.matmul(o_ps[:, 0:66],
                                         qt_feat[0:34, 2, ts(ch, C)],
                                         s_tiles[ch][0:34, 2, 0:66],
                                         start=False, stop=False,
                                         skip_group_check=True)
                        nc.tensor.matmul(o_ps[:, 0:66], at_list[ch],
                                         vha[:, ch, :], start=False, stop=True,
                                         skip_group_check=True)
                    else:
                        nc.tensor.matmul(o_ps[:, 0:66], at_list[ch],
                                         vha[:, ch, :], start=True, stop=True,
                                         skip_group_check=True)
                    # d = q~ . z in column 66
                    for mc in range(2):
                        nc.tensor.matmul(o_ps[:, 66:67],
                                         qt_feat[:, mc, ts(ch, C)],
                                         zcol[:, mc, :], start=(mc == 0),
                                         stop=False, skip_group_check=True)
                    nc.tensor.matmul(o_ps[:, 66:67], qt_feat[0:34, 2, ts(ch, C)],
                                     zcol[0:34, 2, :], start=False, stop=True,
                                     skip_group_check=True)
                    dcols = post.tile([C, 3], F32, tag="dcols")
                    nc.vector.tensor_copy(dcols, o_ps[:, 64:67])
                    dd = post.tile([C, 1], F32, tag="dd")
                    nc.gpsimd.tensor_mul(dd, dcols[:, 0:1], dcols[:, 2:3])
                    rec = post.tile([C, 1], F32, tag="rec")
                    nc.vector.reciprocal(rec, dd)
                    attn_r = post.tile([C, D_V], F32, tag="attn")
                    nc.vector.tensor_scalar_mul(attn_r, o_ps[:, 0:D_V], rec)
                    if debug and ch < 3:
                        ocp = post.tile([C, 67], F32, tag="ocp", name=f"ocp{ch}")
                        nc.vector.tensor_copy(ocp, o_ps)
                        nc.sync.dma_start(out=dbg_o[:, ds(ch * 67, 67)], in_=ocp)
                        nc.sync.dma_start(out=dbg_rec[:, ch:ch + 1], in_=rec)
                        nc.sync.dma_start(out=dbg_att[:, ts(ch, D_V)], in_=attn_r)
                    tr_ps = pstr2.tile([D_V, C], F32, tag="tr")
                    nc.tensor.transpose(tr_ps, attn_r, idf_r)
                    attnT_r = post.tile([D_V, C], BF16, tag="attnT")
                    nc.scalar.copy(attnT_r, tr_ps)
                    if debug and ch < 3:
                        nc.sync.dma_start(out=dbg_attT[:, ts(ch, C)], in_=attnT_r)
                    fc_ps = psfc.tile([C, D_MODEL], F32, tag="fc")
                    nc.tensor.matmul(fc_ps, attnT_r, wfc_r, start=True, stop=True,
                                     skip_group_check=True)
                    if debug and ch < 3:
                        fcp = post.tile([C, D_MODEL], F32, tag="fcp", name=f"fcp{ch}")
                        nc.vector.tensor_copy(fcp, fc_ps)
                        nc.sync.dma_start(out=dbg_fc[:, ds(ch * D_MODEL, D_MODEL)], in_=fcp)
                    o_sb = outp.tile([C, D_MODEL], BF16, tag="osb",
                                     name=f"osb{ch}")
                    eng = (nc.scalar.copy, nc.vector.tensor_copy)[ch % 2]
                    eng(o_sb, fc_ps)
                    nc.sync.dma_start(out=out_d[ts(ch, C), :], in_=o_sb)
    nc.compile()
    return nc


# --------------------------------------------------------------------------
# Host orchestration
# --------------------------------------------------------------------------
_CACHE = {}


def _get_programs():
    if "a" not in _CACHE:
        _CACHE["a"] = build_phase_a()
        _CACHE["b"] = build_phase_b()
    return _CACHE["a"], _CACHE["b"]


def _bf(x):
    return np.ascontiguousarray(np.asarray(x, np.float32).astype(NPBF))


def _prep_a_maps(q, k, v, Wq, Wk, Wv, gamma, beta):
    qT = np.ascontiguousarray(q[0].T)
    kT = np.ascontiguousarray(k[0].T)
    vT = np.ascontiguousarray(v[0].T)
    Wqe = (gamma[:, None] * Wq) * SCALE
    Wke = Wk * SCALE
    cq_all = (beta @ Wq) * SCALE                       # [512]
    cq = np.ascontiguousarray(cq_all.reshape(4, 128).T.astype(np.float32))
    cb = np.zeros((128, 4), np.float32)
    cb[:, 0] = 1.0 / D_MODEL
    cb[:, 1] = 1.0
    cb[0:64, 2] = -0.5
    cb[64:128, 3] = -0.5
    gq = np.concatenate([-Wqe.sum(axis=0), np.ones(128, np.float32)])[None, :]
    misc = np.full((1, 1), LN_EPS, np.float32)
    Wqe_b, Wke_b, Wv_b, cb_b, gq_b = (_bf(Wqe), _bf(Wke), _bf(Wv), _bf(cb),
                                      _bf(gq))
    in_a = []
    for j in range(NC):
        sl = slice(j * SLA, (j + 1) * SLA)
        xs = np.concatenate([qT[:, sl], kT[:, sl], vT[:, sl]], axis=1)
        in_a.append({
            "xs": _bf(xs),
            "Wqe": Wqe_b, "Wke": Wke_b, "Wv": Wv_b,
            "cb": cb_b, "gqneg": gq_b, "cq": cq, "misc": misc,
        })
    return in_a


def _prep_b_maps(W_fc, rf, res_a):
    kh_full = np.concatenate([np.asarray(r["kh"]) for r in res_a], axis=1)
    qh_full = np.concatenate([np.asarray(r["qh"]) for r in res_a], axis=1)
    vh_full = np.concatenate([np.asarray(r["vhT"]) for r in res_a], axis=0)
    k_stab = np.float32(max(float(np.max(r["stab"])) for r in res_a))

    rftT = rf.T.astype(np.float32)                     # [64, 266]
    tri = np.triu(np.ones((C, C), np.float32))
    identm = np.eye(128, dtype=np.float32)
    stabc = np.concatenate([np.full((128, 1), -k_stab, np.float32),
                            np.eye(128, dtype=np.float32)], axis=1)
    eps_row = np.full((1, N), KERNEL_EPS, np.float32)
    misc = np.full((1, 1), M * KERNEL_EPS, np.float32)

    in_b = []
    for h in range(NC):
        rows = slice(h * D_K, (h + 1) * D_K)
        vh_h = np.asarray(vh_full[:, h * D_V:(h + 1) * D_V], np.float32)
        vht = vh_h.reshape(NCH, 128, D_V).transpose(1, 0, 2).reshape(
            128, NCH * D_V)
        blob = np.zeros((128, 128 + 128 + M + M + D_MODEL + 1), np.float32)
        o = 0
        blob[:, o:o + 128] = identm; o += 128
        blob[:, o:o + 128] = tri; o += 128
        blob[0:D_K, o:o + M] = rftT; o += M
        blob[0:D_K, o:o + M] = -0.5; o += M
        blob[0:D_K, o:o + D_MODEL] = W_fc[rows, :] * float(M); o += D_MODEL
        blob[:, o] = 1.0
        in_b.append({
            "khh": _bf(kh_full[rows]),
            "qhh": _bf(qh_full[rows]),
            "vht": _bf(vht),
            "blob": _bf(blob),
            "stabc": stabc, "eps_row": _bf(eps_row), "misc": misc,
        })
    return in_b


def _cast_all(*arrs):
    return [np.asarray(a, np.float32) for a in arrs]


def kernel(q, k, v, Wq, Wk, Wv, W_fc, b_fc, gamma, beta, rf):
    q, k, v, Wq, Wk, Wv, W_fc, b_fc, gamma, beta, rf = _cast_all(
        q, k, v, Wq, Wk, Wv, W_fc, b_fc, gamma, beta, rf)

    nc_a, nc_b = _get_programs()
    cores = list(range(NC))

    in_a = _prep_a_maps(q, k, v, Wq, Wk, Wv, gamma, beta)
    res_a = run_bass_kernel_spmd(nc_a, in_a, core_ids=cores)

    in_b = _prep_b_maps(W_fc, rf, res_a.results)
    res_b = run_bass_kernel_spmd(nc_b, in_b, core_ids=cores)

    out = np.zeros((N, D_MODEL), np.float32)
    for r in res_b.results:
        out += np.asarray(r["out"], np.float32)
    out += b_fc[None, :]
    out += q[0]
    return out[None].astype(np.float32)


def trace_args(inputs):
    """For test.py: returns [(phase, nc, in_maps), ...] re-runnable with trace."""
    q, k, v, Wq, Wk, Wv, W_fc, b_fc, gamma, beta, rf = _cast_all(
        inputs["q"], inputs["k"], inputs["v"], inputs["Wq"], inputs["Wk"],
        inputs["Wv"], inputs["W_fc"], inputs["b_fc"], inputs["gamma"],
        inputs["beta"], inputs["rf"])
    nc_a, nc_b = _get_programs()
    in_a = _prep_a_maps(q, k, v, Wq, Wk, Wv, gamma, beta)
    res_a = run_bass_kernel_spmd(nc_a, in_a, core_ids=list(range(NC)))
    in_b = _prep_b_maps(W_fc, rf, res_a.results)
    return [("a", nc_a, in_a), ("b", nc_b, in_b)]


if __name__ == "__main__":
    rng = np.random.default_rng(0)
    inputs = {
        "q": rng.standard_normal((1, N, D_MODEL)).astype(np.float32),
        "k": rng.standard_normal((1, N, D_MODEL)).astype(np.float32),
        "v": rng.standard_normal((1, N, D_MODEL)).astype(np.float32),
        "Wq": (rng.standard_normal((D_MODEL, 512)) * 0.04).astype(np.float32),
        "Wk": (rng.standard_normal((D_MODEL, 512)) * 0.04).astype(np.float32),
        "Wv": (rng.standard_normal((D_MODEL, 512)) * 0.04).astype(np.float32),
        "W_fc": (rng.standard_normal((512, D_MODEL)) * 0.04).astype(np.float32),
        "b_fc": np.zeros(D_MODEL, np.float32),
        "gamma": np.ones(D_MODEL, np.float32),
        "beta": np.zeros(D_MODEL, np.float32),
        "rf": rng.standard_normal((M, D_K)).astype(np.float32),
    }
    out = kernel(**inputs)
    print("kernel output", out.shape, out.dtype)


# revision 24
# speedup vs baseline: 1.1451x; 1.0580x over previous
"""Performer attention (causal, kernelized) — Trainium2 Bass kernel, v5 (bf16).

Two launches on 8 cores:

  A) seq-sharded prep: core j owns 256 sequence positions and computes, for
     ALL 8 heads at once: kh (scaled k-projection), the LayerNorm-folded and
     scaled/biased q-projection qh, the v-projection in seq-major layout,
     and the local stabilizer max(h_k).

  B) head-sharded attention: core h owns head h end-to-end: Performer
     feature maps, the causal chunked prefix scan (per-chunk states, C=128),
     output normalization and its row-block of the FC (W_fc row-sharded;
     host sums partials and adds bias + residual).

All matmul operands are bfloat16 (PE: 1.0 cycles/row at any free size; DMA
bytes halved; PSUM accumulation stays fp32).  Algebra:
  - q LayerNorm folded: Wq_eff = diag(gamma) Wq * scale, bias cq = beta@Wq*scale,
    applied to (q - mu) * rstd with rstd = rsqrt(var + eps).
  - exp(h_q + (proj_q - h_q)) == exp(proj_q): q-side stabilizer cancels.
  - k feature: exp(proj_k + h_k - k_stab) via the augmented contraction
    [kh; kh^2] . [rf^T; -0.5] plus a constant bias of -k_stab in the exp.
  - the reference's +KERNEL_EPS on both kernels perturbs the attention
    output by ~1e-4 relative; the attention term itself is ~1e-5 of the
    output norm (the reference's double normalization divides by an extra
    q'.sum(k') factor ~1e5), so the eps terms are ~1e-9 of the output and
    are omitted (validated well under the 2e-2 gate).
  - causal prefix scan chunked at C=128 with per-chunk states; diagonal
    128x128 score blocks handle intra-chunk causality via a triangular mask.
  - D (causal normalizer) rides in state/score column 64 (v column 64 == 1);
    d = q~ . z (z = column sums of k~) in o column 66.
  - the reference's |d|<=1e-6 guard is dead for any realistic data and is
    omitted.
"""

import sys
for _p in ("/opt/trn_rl_repo", "/root/.axon_site/_ro/trn_rl_repo"):
    if _p not in sys.path:
        sys.path.append(_p)

import numpy as np
import ml_dtypes

import concourse.bass as bass
from concourse import bacc
import concourse.mybir as mybir
import concourse.tile as tile
from concourse.bass import ts, ds
from concourse.bass_utils import run_bass_kernel_spmd

F32 = mybir.dt.float32
BF16 = mybir.dt.bfloat16
NPBF = ml_dtypes.bfloat16
NC = 8
N = 2048
D_MODEL = 512
D_K = 64
D_V = 64
M = 266
C = 128
NCH = N // C            # 16 chunks
SLA = N // NC           # 256 seq positions per phase-A core
NSL = 4                 # 512-wide slices of the full sequence
SL = 512
KERNEL_EPS = 1e-4
LN_EPS = 1e-6
SCALE = float(D_MODEL) ** (-0.25)
EXP = mybir.ActivationFunctionType.Exp
SQRT = mybir.ActivationFunctionType.Sqrt
IDENT = mybir.ActivationFunctionType.Identity


# --------------------------------------------------------------------------
# Phase A: seq-sharded projections + local stabilizer
# --------------------------------------------------------------------------
def build_phase_a():
    nc = bacc.Bacc("TRN2", target_bir_lowering=False, debug=False, num_devices=NC)
    xs = nc.dram_tensor("xs", [D_MODEL, 3 * SLA], BF16, kind="ExternalInput")
    Wqe = nc.dram_tensor("Wqe", [D_MODEL, D_MODEL], BF16, kind="ExternalInput")
    Wke = nc.dram_tensor("Wke", [D_MODEL, D_MODEL], BF16, kind="ExternalInput")
    Wv = nc.dram_tensor("Wv", [D_MODEL, D_MODEL], BF16, kind="ExternalInput")
    # bf16 consts [128, 4]: wmean | ones | neghalf head-lo | neghalf head-hi
    cb = nc.dram_tensor("cb", [128, 4], BF16, kind="ExternalInput")
    # row consts [1, 640] bf16: -sum(Wqe) (512) | ones (128)
    gqon = nc.dram_tensor("gqon", [1, D_MODEL + 128], BF16, kind="ExternalInput")
    # f32 consts [128, 5]: cq (4 cols) | col4 row0 = LN_EPS
    cqm = nc.dram_tensor("cqm", [128, 5], F32, kind="ExternalInput")
    kh_out = nc.dram_tensor("kh", [D_MODEL, SLA], BF16, kind="ExternalOutput")
    qh_out = nc.dram_tensor("qh", [D_MODEL, SLA], BF16, kind="ExternalOutput")
    vhT_out = nc.dram_tensor("vhT", [SLA, D_MODEL], BF16, kind="ExternalOutput")
    stab_out = nc.dram_tensor("stab", [2, 1], F32, kind="ExternalOutput")

    with tile.TileContext(nc) as tc:
        with (
            tc.tile_pool(name="wts", bufs=1) as wts,
            tc.tile_pool(name="xin", bufs=1) as xin,
            tc.tile_pool(name="work", bufs=1) as work,
            tc.tile_pool(name="stat", bufs=1) as statp,
            tc.tile_pool(name="outs", bufs=1) as outs,
        ):
            # ---- loads: consts first (tiny), then Wk+k (kh path), q (LN),
            # v, Wq, Wv
            cb_sb = wts.tile([128, 4], BF16)
            nc.scalar.dma_start(out=cb_sb, in_=cb[:, :])
            cqm_sb = wts.tile([128, 5], F32)
            nc.scalar.dma_start(out=cqm_sb, in_=cqm[:, :])
            gq_sb = wts.tile([1, D_MODEL + 128], BF16)
            nc.scalar.dma_start(out=gq_sb, in_=gqon[:, :])
            wk_r = wts.tile([128, 4, D_MODEL], BF16)
            nc.sync.dma_start(out=wk_r, in_=Wke[:, :].rearrange("(c p) f -> p c f", p=128))
            k_r = xin.tile([128, 4, SLA], BF16)
            nc.sync.dma_start(out=k_r, in_=xs[:, SLA:2 * SLA].rearrange("(c p) f -> p c f", p=128))
            q_r = xin.tile([128, 4, SLA], BF16)
            nc.sync.dma_start(out=q_r, in_=xs[:, 0:SLA].rearrange("(c p) f -> p c f", p=128))
            v_r = xin.tile([128, 4, SLA], BF16)
            nc.sync.dma_start(out=v_r, in_=xs[:, 2 * SLA:3 * SLA].rearrange("(c p) f -> p c f", p=128))
            wq_r = wts.tile([128, 4, D_MODEL], BF16)
            nc.sync.dma_start(out=wq_r, in_=Wqe[:, :].rearrange("(c p) f -> p c f", p=128))
            wv_r = wts.tile([128, 4, D_MODEL], BF16)
            nc.gpsimd.dma_start(out=wv_r, in_=Wv[:, :].rearrange("(c p) f -> p c f", p=128))

            wm_r = cb_sb[:, 0:1]      # 1/512
            nh_r = cb_sb[:, 2:4]      # -0.5 split into per-head halves
            cq_sb = cqm_sb[:, 0:4]
            eps_sb = cqm_sb[0:1, 4:5]

            def q_c(c):
                return q_r[:, c, :]

            def k_c(c):
                return k_r[:, c, :]

            def v_c(c):
                return v_r[:, c, :]

            # ---- LayerNorm stats on q (over d_model, per position) ----
            mu_r = statp.tile([1, SLA], BF16)
            with tc.tile_pool(name="pss", bufs=1, space="PSUM") as pss:
                mu_ps = pss.tile([1, SLA], F32, tag="mu")
                for c in range(4):
                    nc.tensor.matmul(mu_ps, wm_r, q_c(c), start=(c == 0),
                                     stop=(c == 3), skip_group_check=True)
                qsq_r = work.tile([128, 4, SLA], BF16)
                nc.vector.tensor_mul(qsq_r, q_r, q_r)
                msq_ps = pss.tile([1, SLA], F32, tag="msq")
                for c in range(4):
                    nc.tensor.matmul(msq_ps, wm_r, qsq_r[:, c, :], start=(c == 0),
                                     stop=(c == 3), skip_group_check=True)
                mu_f = statp.tile([1, SLA], F32)
                nc.vector.tensor_copy(mu_f, mu_ps)
                nc.vector.tensor_copy(mu_r, mu_ps)
                var_sb = statp.tile([1, SLA], F32)
                nc.vector.tensor_mul(var_sb, mu_f, mu_f)
                nc.vector.tensor_sub(var_sb, msq_ps, var_sb)
                srt_sb = statp.tile([1, SLA], F32)
                nc.scalar.activation(srt_sb, var_sb, SQRT,
                                     bias=eps_sb, scale=1.0)
                rstd_r = statp.tile([1, SLA], BF16)
                with nc.allow_low_precision(reason="bf16 layernorm scale"):
                    nc.vector.reciprocal(rstd_r, srt_sb)

            # ---- projections: kh first (only needs Wk), then qh, then vh ----
            kh_sb = outs.tile([128, 4, SLA], BF16)
            kh2_r = work.tile([128, 4, SLA], BF16)
            qh_sb = outs.tile([128, 4, SLA], BF16)
            vhT_sb = outs.tile([128, 2, D_MODEL], BF16)
            with tc.tile_pool(name="psb", bufs=2, space="PSUM") as psb:
                for oc in range(4):
                    kh_ps = psb.tile([128, SLA], F32, tag="kh")
                    for c in range(4):
                        nc.tensor.matmul(kh_ps, wk_r[:, c, ts(oc, 128)],
                                         k_c(c), start=(c == 0),
                                         stop=(c == 3), skip_group_check=True)
                    nc.scalar.copy(kh_sb[:, oc, :], kh_ps)
                    nc.vector.tensor_mul(kh2_r[:, oc, :], kh_sb[:, oc, :],
                                         kh_sb[:, oc, :])
                nc.sync.dma_start(
                    out=kh_out[:, :].rearrange("(c p) f -> p c f", p=128),
                    in_=kh_sb)

                # local stabilizer max over heads/positions of -0.5||kh_h||^2;
                # each 128-partition oc chunk holds 2 heads (64 dims each)
                hkm = statp.tile([2, 4], F32)
                for oc in range(4):
                    hk_ps = psb.tile([2, SLA], F32, tag="hk", bufs=1,
                                     name=f"hk{oc}")
                    nc.tensor.matmul(hk_ps, nh_r, kh2_r[:, oc, :],
                                     start=True, stop=True,
                                     skip_group_check=True)
                    nc.vector.reduce_max(hkm[:, oc:oc + 1], hk_ps,
                                         axis=mybir.AxisListType.X)
                stab_sb = statp.tile([2, 1], F32)
                nc.vector.reduce_max(stab_sb, hkm, axis=mybir.AxisListType.X)
                nc.scalar.dma_start(out=stab_out[:, :], in_=stab_sb)

                # rstd broadcast to 128 partitions
                rsbc_sb = work.tile([128, SLA], BF16)
                rsbc_ps = psb.tile([128, SLA], F32, tag="rsbc", bufs=1)
                nc.tensor.matmul(rsbc_ps, gq_sb[0:1, D_MODEL:D_MODEL + 128],
                                 rstd_r, start=True, stop=True,
                                 skip_group_check=True)
                nc.scalar.copy(rsbc_sb, rsbc_ps)

                for oc in range(4):
                    qh_ps = psb.tile([128, SLA], F32, tag="qh")
                    for c in range(4):
                        nc.tensor.matmul(qh_ps, wq_r[:, c, ts(oc, 128)],
                                         q_c(c), start=(c == 0),
                                         stop=False, skip_group_check=True)
                    nc.tensor.matmul(qh_ps, gq_sb[0:1, ts(oc, 128)], mu_r,
                                     start=False, stop=True,
                                     skip_group_check=True)
                    nc.vector.tensor_mul(qh_sb[:, oc, :], qh_ps, rsbc_sb)
                    nc.scalar.activation(qh_sb[:, oc, :], qh_sb[:, oc, :], IDENT,
                                         bias=cq_sb[:, oc:oc + 1], scale=1.0)
                nc.sync.dma_start(
                    out=qh_out[:, :].rearrange("(c p) f -> p c f", p=128),
                    in_=qh_sb)

                for sc in range(2):
                    vh_ps = psb.tile([128, D_MODEL], F32, tag="vh")
                    for c in range(4):
                        nc.tensor.matmul(vh_ps, v_c(c)[:, ts(sc, 128)],
                                         wv_r[:, c, :], start=(c == 0),
                                         stop=(c == 3), skip_group_check=True)
                    if sc == 0:
                        nc.scalar.copy(vhT_sb[:, sc, :], vh_ps)
                    else:
                        nc.vector.tensor_copy(vhT_sb[:, sc, :], vh_ps)
                nc.sync.dma_start(
                    out=vhT_out[:, :].rearrange("(s p) f -> p s f", p=128),
                    in_=vhT_sb)
    nc.compile()
    return nc


# --------------------------------------------------------------------------
# Phase B: head-sharded Performer attention + FC row-block
# --------------------------------------------------------------------------
def build_phase_b(debug=False):
    nc = bacc.Bacc("TRN2", target_bir_lowering=False, debug=False, num_devices=NC)
    khh = nc.dram_tensor("khh", [D_K, N], BF16, kind="ExternalInput")
    qhh = nc.dram_tensor("qhh", [D_K, N], BF16, kind="ExternalInput")
    vht = nc.dram_tensor("vht", [128, NCH * D_V], BF16, kind="ExternalInput")
    # bf16 const blob [128, 128+128+266+266+512]: idm|tri|rft|rneg|wfc
    NBLOB = 128 + 128 + M + M + D_MODEL
    blob = nc.dram_tensor("blob", [128, NBLOB], BF16, kind="ExternalInput")
    # f32 consts [128, 129]: col0 = -k_stab, cols 1:129 identity (transpose)
    stabc = nc.dram_tensor("stabc", [128, 129], F32, kind="ExternalInput")
    out_d = nc.dram_tensor("out", [N, D_MODEL], BF16, kind="ExternalOutput")

    with tile.TileContext(nc) as tc:
        with (
            tc.tile_pool(name="consts", bufs=1) as consts,
            tc.tile_pool(name="krows", bufs=1) as krows,
            tc.tile_pool(name="feat", bufs=1) as feat,
            tc.tile_pool(name="ktrp", bufs=1) as ktrp,
            tc.tile_pool(name="ktT", bufs=NCH) as ktTp,
            tc.tile_pool(name="atp", bufs=NCH) as atp,
            tc.tile_pool(name="ssb", bufs=NCH + 1) as ssbp,
            tc.tile_pool(name="post", bufs=4) as post,
            tc.tile_pool(name="outp", bufs=3) as outp,
        ):
            # ---- loads ----
            blob_sb = consts.tile([128, NBLOB], BF16)
            nc.scalar.dma_start(out=blob_sb, in_=blob[:, :])
            stab_full = consts.tile([128, 129], F32)
            nc.scalar.dma_start(out=stab_full, in_=stabc[:, :])
            stab_sb = stab_full[:, 0:1]
            idf_r = stab_full[:, 1:129]
            qhr = krows.tile([D_K, N], BF16)
            nc.sync.dma_start(out=qhr, in_=qhh[:, :])
            khr = krows.tile([D_K, N], BF16)
            nc.sync.dma_start(out=khr, in_=khh[:, :])
            vha = krows.tile([128, NCH, 66], BF16)
            nc.gpsimd.dma_start(
                out=vha[:, :, 0:D_V],
                in_=vht[:, :].rearrange("p (ch f) -> p ch f", ch=NCH))

            id_r = blob_sb[:, 0:128]
            tri_r = blob_sb[:, 128:256]
            rft_r = blob_sb[0:D_K, 256:256 + M]
            rneg_r = blob_sb[0:D_K, 256 + M:256 + 2 * M]
            wfc_r = blob_sb[0:D_V, 256 + 2 * M:256 + 2 * M + D_MODEL]

            # vha constant columns: 64 -> 1.0, 65 -> 0.0
            nc.gpsimd.memset(vha[:, :, D_V:D_V + 1], 1.0)
            nc.gpsimd.memset(vha[:, :, D_V + 1:D_V + 2], 0.0)

            # kh^2 rows
            kh2r = krows.tile([D_K, N], BF16)
            nc.vector.tensor_mul(kh2r, khr, khr)

            # q~ features m-major [128, 3, N]; mc2 = rows 0..9
            qt_feat = feat.tile([128, 3, N], BF16)
            # k~ features seq-major [C, ch, 266]
            ktr = ktrp.tile([128, NCH, M], BF16)

            s_tiles = [ssbp.tile([128, 3, 66], BF16, tag="ssb", name=f"ssb{i}")
                       for i in range(NCH + 1)]

            ktT_tiles = {}
            at_list = []
            with (
                tc.tile_pool(name="psq", bufs=2, space="PSUM") as psq,
                tc.tile_pool(name="pskp", bufs=2, space="PSUM") as pskp,
                tc.tile_pool(name="pstr", bufs=2, space="PSUM") as pstr,
                tc.tile_pool(name="psat", bufs=1, space="PSUM") as psat,
                tc.tile_pool(name="pssd", bufs=1, space="PSUM") as pssd,
            ):
              for s in range(NSL):
                    # ---- q features for slice s ----
                    for mc in range(3):
                        mrows = 128 if mc < 2 else 10
                        qp_ps = psq.tile([128, SL], F32, tag="qp")
                        nc.tensor.matmul(
                            qp_ps[0:mrows, :], rft_r[:, ds(mc * 128, mrows)],
                            qhr[:, ts(s, SL)], start=True, stop=True,
                            skip_group_check=True)
                        nc.scalar.activation(
                            qt_feat[0:mrows, mc, ts(s, SL)],
                            qp_ps[0:mrows, :], EXP, bias=0.0, scale=1.0)

                    # ---- k features + scan for the 4 chunks of slice s ----
                    for ch in range(4 * s, 4 * s + 4):
                        kp_ps = pskp.tile([C, M], F32, tag="kp")
                        nc.tensor.matmul(kp_ps, khr[:, ts(ch, C)], rft_r,
                                         start=True, stop=False,
                                         skip_group_check=True)
                        nc.tensor.matmul(kp_ps, kh2r[:, ts(ch, C)], rneg_r,
                                         start=False, stop=True,
                                         skip_group_check=True)
                        with nc.allow_low_precision(reason="bf16 features"):
                            nc.scalar.activation(
                                ktr[:, ch, 0:M], kp_ps, EXP,
                                bias=stab_sb, scale=1.0)
                        # transpose k~ chunk to m-major
                        tp_ps = pstr.tile([128, 3, 128], BF16, tag="tp")
                        nc.tensor.transpose(tp_ps[:, 0, :], ktr[:, ch, 0:128], id_r)
                        nc.tensor.transpose(tp_ps[:, 1, :], ktr[:, ch, 128:256], id_r)
                        nc.tensor.transpose(tp_ps[0:10, 2, :], ktr[:, ch, 256:266], id_r)
                        ktT = ktTp.tile([128, 3, C], BF16, tag="ktT", name=f"ktT{ch}")
                        nc.scalar.copy(ktT[:, 0, :], tp_ps[:, 0, :])
                        nc.vector.tensor_copy(ktT[:, 1, :], tp_ps[:, 1, :])
                        nc.scalar.copy(ktT[0:10, 2, :], tp_ps[0:10, 2, :])
                        ktT_tiles[ch] = ktT

                        # diagonal score block (keys ch x queries ch), masked
                        at_ps = psat.tile([C, C], F32, tag="at")
                        for mc in range(2):
                            nc.tensor.matmul(at_ps, ktT[:, mc, :],
                                             qt_feat[:, mc, ts(ch, C)],
                                             start=(mc == 0), stop=False,
                                             skip_group_check=True)
                        nc.tensor.matmul(at_ps, ktT[0:10, 2, :],
                                         qt_feat[0:10, 2, ts(ch, C)],
                                         start=False, stop=True,
                                         skip_group_check=True)
                        at_r = atp.tile([C, C], BF16, tag="at_r", name=f"atr{ch}")
                        nc.vector.tensor_mul(at_r, at_ps, tri_r)
                        at_list.append(at_r)

                        # state update for this chunk
                        sd_ps = pssd.tile([128, 3, 66], F32, tag="sd")
                        if ch == 0:
                            # one-time zero of mc2 rows the matmuls never
                            # write; persists across psum reuse (bufs=1)
                            nc.vector.memset(sd_ps[:, 2, :], 0.0)
                        for mc, cols in ((0, (0, 128)), (1, (128, 256)),
                                         (2, (256, 266))):
                            dst = sd_ps[:, mc, :] if mc < 2 else sd_ps[0:10, 2, :]
                            nc.tensor.matmul(dst, ktr[:, ch, cols[0]:cols[1]],
                                             vha[:, ch, 0:66], start=True,
                                             stop=True, skip_group_check=True)
                        if ch == 0:
                            nc.vector.tensor_copy(s_tiles[1][:, :, :], sd_ps)
                        else:
                            nc.vector.tensor_add(s_tiles[ch + 1][:, :, :],
                                                 s_tiles[ch][:, :, :], sd_ps)

            # z column = final state's column 64
            zcol = feat.tile([128, 3, 1], BF16)
            nc.vector.tensor_copy(zcol, s_tiles[NCH][:, :, 64:65])

            # ---- per-chunk output, normalize, FC ----
            with (
                tc.tile_pool(name="pso", bufs=3, space="PSUM") as pso,
                tc.tile_pool(name="psfc", bufs=2, space="PSUM") as psfc,
                tc.tile_pool(name="pstr2", bufs=2, space="PSUM") as pstr2,
            ):
                for ch in range(NCH):
                    o_ps = pso.tile([C, 67], F32, tag="o")
                    if ch > 0:
                        for mc in range(2):
                            nc.tensor.matmul(o_ps[:, 0:66],
                                             qt_feat[:, mc, ts(ch, C)],
                                             s_tiles[ch][:, mc, 0:66],
                                             start=(mc == 0), stop=False,
                                             skip_group_check=True)
                        nc.tensor.matmul(o_ps[:, 0:66],
                                         qt_feat[0:10, 2, ts(ch, C)],
                                         s_tiles[ch][0:10, 2, 0:66],
                                         start=False, stop=False,
                                         skip_group_check=True)
                        nc.tensor.matmul(o_ps[:, 0:66], at_list[ch],
                                         vha[:, ch, :], start=False, stop=True,
                                         skip_group_check=True)
                    else:
                        nc.tensor.matmul(o_ps[:, 0:66], at_list[ch],
                                         vha[:, ch, :], start=True, stop=True,
                                         skip_group_check=True)
                    # d = q~ . z in column 66
                    for mc in range(2):
                        nc.tensor.matmul(o_ps[:, 66:67],
                                         qt_feat[:, mc, ts(ch, C)],
                                         zcol[:, mc, :], start=(mc == 0),
                                         stop=False, skip_group_check=True)
                    nc.tensor.matmul(o_ps[:, 66:67], qt_feat[0:10, 2, ts(ch, C)],
                                     zcol[0:10, 2, :], start=False, stop=True,
                                     skip_group_check=True)
                    # normalization tail kept on one engine (program-order
                    # chaining, no cross-engine semaphore hops)
                    dcols = post.tile([C, 3], F32, tag="dcols")
                    nc.vector.tensor_copy(dcols, o_ps[:, 64:67])
                    dd = post.tile([C, 1], F32, tag="dd")
                    nc.vector.tensor_mul(dd, dcols[:, 0:1], dcols[:, 2:3])
                    rec = post.tile([C, 1], F32, tag="rec")
                    nc.vector.reciprocal(rec, dd)
                    attn_r = post.tile([C, D_V], F32, tag="attn")
                    nc.vector.tensor_scalar_mul(attn_r, o_ps[:, 0:D_V], rec)
                    tr_ps = pstr2.tile([D_V, C], F32, tag="tr")
                    nc.tensor.transpose(tr_ps, attn_r, idf_r)
                    attnT_r = post.tile([D_V, C], BF16, tag="attnT")
                    nc.scalar.copy(attnT_r, tr_ps)
                    fc_ps = psfc.tile([C, D_MODEL], F32, tag="fc")
                    nc.tensor.matmul(fc_ps, attnT_r, wfc_r, start=True, stop=True,
                                     skip_group_check=True)
                    o_sb = outp.tile([C, D_MODEL], BF16, tag="osb",
                                     name=f"osb{ch}")
                    eng = (nc.scalar.copy, nc.vector.tensor_copy)[ch % 2]
                    eng(o_sb, fc_ps)
                    nc.sync.dma_start(out=out_d[ts(ch, C), :], in_=o_sb)
    nc.compile()
    return nc


# --------------------------------------------------------------------------
# Host orchestration
# --------------------------------------------------------------------------
_CACHE = {}


def _get_programs():
    if "a" not in _CACHE:
        _CACHE["a"] = build_phase_a()
        _CACHE["b"] = build_phase_b()
    return _CACHE["a"], _CACHE["b"]


def _bf(x):
    return np.ascontiguousarray(np.asarray(x, np.float32).astype(NPBF))


def _prep_a_maps(q, k, v, Wq, Wk, Wv, gamma, beta):
    qT = np.ascontiguousarray(q[0].T)
    kT = np.ascontiguousarray(k[0].T)
    vT = np.ascontiguousarray(v[0].T)
    Wqe = (gamma[:, None] * Wq) * SCALE
    Wke = Wk * SCALE
    cq_all = (beta @ Wq) * SCALE                       # [512]
    cqm = np.zeros((128, 5), np.float32)
    cqm[:, 0:4] = cq_all.reshape(4, 128).T
    cqm[0, 4] = LN_EPS
    cb = np.zeros((128, 4), np.float32)
    cb[:, 0] = 1.0 / D_MODEL
    cb[:, 1] = 1.0
    cb[0:64, 2] = -0.5
    cb[64:128, 3] = -0.5
    gqon = np.concatenate([-Wqe.sum(axis=0),
                           np.ones(128, np.float32)])[None, :]
    Wqe_b, Wke_b, Wv_b, cb_b, gq_b = (_bf(Wqe), _bf(Wke), _bf(Wv), _bf(cb),
                                      _bf(gqon))
    in_a = []
    for j in range(NC):
        sl = slice(j * SLA, (j + 1) * SLA)
        xs = np.concatenate([qT[:, sl], kT[:, sl], vT[:, sl]], axis=1)
        in_a.append({
            "xs": _bf(xs),
            "Wqe": Wqe_b, "Wke": Wke_b, "Wv": Wv_b,
            "cb": cb_b, "gqon": gq_b, "cqm": cqm,
        })
    return in_a


def _prep_b_maps(W_fc, rf, res_a):
    kh_full = np.concatenate([np.asarray(r["kh"]) for r in res_a], axis=1)
    qh_full = np.concatenate([np.asarray(r["qh"]) for r in res_a], axis=1)
    vh_full = np.concatenate([np.asarray(r["vhT"]) for r in res_a], axis=0)
    k_stab = np.float32(max(float(np.max(r["stab"])) for r in res_a))

    rftT = rf.T.astype(np.float32)                     # [64, 266]
    tri = np.triu(np.ones((C, C), np.float32))
    identm = np.eye(128, dtype=np.float32)
    stabc = np.concatenate([np.full((128, 1), -k_stab, np.float32),
                            identm], axis=1)

    in_b = []
    for h in range(NC):
        rows = slice(h * D_K, (h + 1) * D_K)
        vh_h = np.asarray(vh_full[:, h * D_V:(h + 1) * D_V], np.float32)
        vht = vh_h.reshape(NCH, 128, D_V).transpose(1, 0, 2).reshape(
            128, NCH * D_V)
        blob = np.zeros((128, 128 + 128 + M + M + D_MODEL), np.float32)
        o = 0
        blob[:, o:o + 128] = identm; o += 128
        blob[:, o:o + 128] = tri; o += 128
        blob[0:D_K, o:o + M] = rftT; o += M
        blob[0:D_K, o:o + M] = -0.5; o += M
        blob[0:D_K, o:o + D_MODEL] = W_fc[rows, :] * float(M)
        in_b.append({
            "khh": _bf(kh_full[rows]),
            "qhh": _bf(qh_full[rows]),
            "vht": _bf(vht),
            "blob": _bf(blob),
            "stabc": stabc,
        })
    return in_b


def _cast_all(*arrs):
    return [np.asarray(a, np.float32) for a in arrs]


def kernel(q, k, v, Wq, Wk, Wv, W_fc, b_fc, gamma, beta, rf):
    q, k, v, Wq, Wk, Wv, W_fc, b_fc, gamma, beta, rf = _cast_all(
        q, k, v, Wq, Wk, Wv, W_fc, b_fc, gamma, beta, rf)

    nc_a, nc_b = _get_programs()
    cores = list(range(NC))

    in_a = _prep_a_maps(q, k, v, Wq, Wk, Wv, gamma, beta)
    res_a = run_bass_kernel_spmd(nc_a, in_a, core_ids=cores)

    in_b = _prep_b_maps(W_fc, rf, res_a.results)
    res_b = run_bass_kernel_spmd(nc_b, in_b, core_ids=cores)

    out = np.zeros((N, D_MODEL), np.float32)
    for r in res_b.results:
        out += np.asarray(r["out"], np.float32)
    out += b_fc[None, :]
    out += q[0]
    return out[None].astype(np.float32)


def trace_args(inputs):
    """For test.py: returns [(phase, nc, in_maps), ...] re-runnable with trace."""
    q, k, v, Wq, Wk, Wv, W_fc, b_fc, gamma, beta, rf = _cast_all(
        inputs["q"], inputs["k"], inputs["v"], inputs["Wq"], inputs["Wk"],
        inputs["Wv"], inputs["W_fc"], inputs["b_fc"], inputs["gamma"],
        inputs["beta"], inputs["rf"])
    nc_a, nc_b = _get_programs()
    in_a = _prep_a_maps(q, k, v, Wq, Wk, Wv, gamma, beta)
    res_a = run_bass_kernel_spmd(nc_a, in_a, core_ids=list(range(NC)))
    in_b = _prep_b_maps(W_fc, rf, res_a.results)
    return [("a", nc_a, in_a), ("b", nc_b, in_b)]


if __name__ == "__main__":
    rng = np.random.default_rng(0)
    inputs = {
        "q": rng.standard_normal((1, N, D_MODEL)).astype(np.float32),
        "k": rng.standard_normal((1, N, D_MODEL)).astype(np.float32),
        "v": rng.standard_normal((1, N, D_MODEL)).astype(np.float32),
        "Wq": (rng.standard_normal((D_MODEL, 512)) * 0.04).astype(np.float32),
        "Wk": (rng.standard_normal((D_MODEL, 512)) * 0.04).astype(np.float32),
        "Wv": (rng.standard_normal((D_MODEL, 512)) * 0.04).astype(np.float32),
        "W_fc": (rng.standard_normal((512, D_MODEL)) * 0.04).astype(np.float32),
        "b_fc": np.zeros(D_MODEL, np.float32),
        "gamma": np.ones(D_MODEL, np.float32),
        "beta": np.zeros(D_MODEL, np.float32),
        "rf": rng.standard_normal((M, D_K)).astype(np.float32),
    }
    out = kernel(**inputs)
    print("kernel output", out.shape, out.dtype)


# revision 25
# speedup vs baseline: 1.2350x; 1.0785x over previous
"""Performer attention (causal, kernelized) — Trainium2 Bass kernel, v5 (bf16).

Two launches on 8 cores:

  A) seq-sharded prep: core j owns 256 sequence positions and computes, for
     ALL 8 heads at once: kh (scaled k-projection), the LayerNorm-folded and
     scaled/biased q-projection qh, the v-projection in seq-major layout,
     and the local stabilizer max(h_k).

  B) head-sharded attention: core h owns head h end-to-end: Performer
     feature maps, the causal chunked prefix scan (per-chunk states, C=128),
     output normalization and its row-block of the FC (W_fc row-sharded;
     host sums partials and adds bias + residual).

All matmul operands are bfloat16 (PE: 1.0 cycles/row at any free size; DMA
bytes halved; PSUM accumulation stays fp32).  Algebra:
  - q LayerNorm folded: Wq_eff = diag(gamma) Wq * scale, bias cq = beta@Wq*scale,
    applied to (q - mu) * rstd with rstd = rsqrt(var + eps).
  - exp(h_q + (proj_q - h_q)) == exp(proj_q): q-side stabilizer cancels.
  - k feature: exp(proj_k + h_k - k_stab) via the augmented contraction
    [kh; kh^2] . [rf^T; -0.5] plus a constant bias of -k_stab in the exp.
  - the reference's +KERNEL_EPS on both kernels perturbs the attention
    output by ~1e-4 relative; the attention term itself is ~1e-5 of the
    output norm (the reference's double normalization divides by an extra
    q'.sum(k') factor ~1e5), so the eps terms are ~1e-9 of the output and
    are omitted (validated well under the 2e-2 gate).
  - causal prefix scan chunked at C=128 with per-chunk states; diagonal
    128x128 score blocks handle intra-chunk causality via a triangular mask.
  - D (causal normalizer) rides in state/score column 64 (v column 64 == 1);
    d = q~ . z (z = column sums of k~) in o column 66.
  - the reference's |d|<=1e-6 guard is dead for any realistic data and is
    omitted.
"""

import sys
for _p in ("/opt/trn_rl_repo", "/root/.axon_site/_ro/trn_rl_repo"):
    if _p not in sys.path:
        sys.path.append(_p)

import numpy as np
import ml_dtypes

import concourse.bass as bass
from concourse import bacc
import concourse.mybir as mybir
import concourse.tile as tile
from concourse.bass import ts, ds
from concourse.bass_utils import run_bass_kernel_spmd

F32 = mybir.dt.float32
BF16 = mybir.dt.bfloat16
NPBF = ml_dtypes.bfloat16
NC = 8
N = 2048
D_MODEL = 512
D_K = 64
D_V = 64
M = 266
C = 128
NCH = N // C            # 16 chunks
SLA = N // NC           # 256 seq positions per phase-A core
NSL = 4                 # 512-wide slices of the full sequence
SL = 512
KERNEL_EPS = 1e-4
LN_EPS = 1e-6
SCALE = float(D_MODEL) ** (-0.25)
EXP = mybir.ActivationFunctionType.Exp
SQRT = mybir.ActivationFunctionType.Sqrt
IDENT = mybir.ActivationFunctionType.Identity


# --------------------------------------------------------------------------
# Phase A: seq-sharded projections + local stabilizer
# --------------------------------------------------------------------------
def build_phase_a():
    nc = bacc.Bacc("TRN2", target_bir_lowering=False, debug=False, num_devices=NC)
    xs = nc.dram_tensor("xs", [D_MODEL, 3 * SLA], BF16, kind="ExternalInput")
    Wqe = nc.dram_tensor("Wqe", [D_MODEL, D_MODEL], BF16, kind="ExternalInput")
    Wke = nc.dram_tensor("Wke", [D_MODEL, D_MODEL], BF16, kind="ExternalInput")
    Wv = nc.dram_tensor("Wv", [D_MODEL, D_MODEL], BF16, kind="ExternalInput")
    # bf16 consts [128, 4]: wmean | ones | neghalf head-lo | neghalf head-hi
    cb = nc.dram_tensor("cb", [128, 4], BF16, kind="ExternalInput")
    # row consts [1, 640] bf16: -sum(Wqe) (512) | ones (128)
    gqon = nc.dram_tensor("gqon", [1, D_MODEL + 128], BF16, kind="ExternalInput")
    # f32 consts [128, 5]: cq (4 cols) | col4 row0 = LN_EPS
    cqm = nc.dram_tensor("cqm", [128, 5], F32, kind="ExternalInput")
    kh_out = nc.dram_tensor("kh", [D_MODEL, SLA], BF16, kind="ExternalOutput")
    qh_out = nc.dram_tensor("qh", [D_MODEL, SLA], BF16, kind="ExternalOutput")
    vhT_out = nc.dram_tensor("vhT", [SLA, D_MODEL], BF16, kind="ExternalOutput")
    stab_out = nc.dram_tensor("stab", [2, 1], F32, kind="ExternalOutput")

    with tile.TileContext(nc) as tc:
        with (
            tc.tile_pool(name="wts", bufs=1) as wts,
            tc.tile_pool(name="xin", bufs=1) as xin,
            tc.tile_pool(name="work", bufs=1) as work,
            tc.tile_pool(name="stat", bufs=1) as statp,
            tc.tile_pool(name="outs", bufs=1) as outs,
        ):
            # ---- loads: consts first (tiny), then Wk+k (kh path), q (LN),
            # v, Wq, Wv
            cb_sb = wts.tile([128, 4], BF16)
            nc.scalar.dma_start(out=cb_sb, in_=cb[:, :])
            cqm_sb = wts.tile([128, 5], F32)
            nc.scalar.dma_start(out=cqm_sb, in_=cqm[:, :])
            gq_sb = wts.tile([1, D_MODEL + 128], BF16)
            nc.scalar.dma_start(out=gq_sb, in_=gqon[:, :])
            wk_r = wts.tile([128, 4, D_MODEL], BF16)
            nc.sync.dma_start(out=wk_r, in_=Wke[:, :].rearrange("(c p) f -> p c f", p=128))
            k_r = xin.tile([128, 4, SLA], BF16)
            nc.sync.dma_start(out=k_r, in_=xs[:, SLA:2 * SLA].rearrange("(c p) f -> p c f", p=128))
            q_r = xin.tile([128, 4, SLA], BF16)
            nc.sync.dma_start(out=q_r, in_=xs[:, 0:SLA].rearrange("(c p) f -> p c f", p=128))
            v_r = xin.tile([128, 4, SLA], BF16)
            nc.sync.dma_start(out=v_r, in_=xs[:, 2 * SLA:3 * SLA].rearrange("(c p) f -> p c f", p=128))
            wq_r = wts.tile([128, 4, D_MODEL], BF16)
            nc.sync.dma_start(out=wq_r, in_=Wqe[:, :].rearrange("(c p) f -> p c f", p=128))
            wv_r = wts.tile([128, 4, D_MODEL], BF16)
            nc.gpsimd.dma_start(out=wv_r, in_=Wv[:, :].rearrange("(c p) f -> p c f", p=128))

            wm_r = cb_sb[:, 0:1]      # 1/512
            nh_r = cb_sb[:, 2:4]      # -0.5 split into per-head halves
            cq_sb = cqm_sb[:, 0:4]
            eps_sb = cqm_sb[0:1, 4:5]

            def q_c(c):
                return q_r[:, c, :]

            def k_c(c):
                return k_r[:, c, :]

            def v_c(c):
                return v_r[:, c, :]

            # ---- LayerNorm stats on q (over d_model, per position) ----
            mu_r = statp.tile([1, SLA], BF16)
            with tc.tile_pool(name="pss", bufs=1, space="PSUM") as pss:
                mu_ps = pss.tile([1, SLA], F32, tag="mu")
                for c in range(4):
                    nc.tensor.matmul(mu_ps, wm_r, q_c(c), start=(c == 0),
                                     stop=(c == 3), skip_group_check=True)
                qsq_r = work.tile([128, 4, SLA], BF16)
                nc.vector.tensor_mul(qsq_r, q_r, q_r)
                msq_ps = pss.tile([1, SLA], F32, tag="msq")
                for c in range(4):
                    nc.tensor.matmul(msq_ps, wm_r, qsq_r[:, c, :], start=(c == 0),
                                     stop=(c == 3), skip_group_check=True)
                mu_f = statp.tile([1, SLA], F32)
                nc.vector.tensor_copy(mu_f, mu_ps)
                nc.vector.tensor_copy(mu_r, mu_ps)
                var_sb = statp.tile([1, SLA], F32)
                nc.vector.tensor_mul(var_sb, mu_f, mu_f)
                nc.vector.tensor_sub(var_sb, msq_ps, var_sb)
                srt_sb = statp.tile([1, SLA], F32)
                nc.scalar.activation(srt_sb, var_sb, SQRT,
                                     bias=eps_sb, scale=1.0)
                rstd_r = statp.tile([1, SLA], BF16)
                with nc.allow_low_precision(reason="bf16 layernorm scale"):
                    nc.vector.reciprocal(rstd_r, srt_sb)

            # ---- projections: kh first (only needs Wk), then qh, then vh ----
            kh_sb = outs.tile([128, 4, SLA], BF16)
            kh2_r = work.tile([128, 4, SLA], BF16)
            qh_sb = outs.tile([128, 4, SLA], BF16)
            vhT_sb = outs.tile([128, 2, D_MODEL], BF16)
            with tc.tile_pool(name="psb", bufs=2, space="PSUM") as psb:
                for oc in range(4):
                    kh_ps = psb.tile([128, SLA], F32, tag="kh")
                    for c in range(4):
                        nc.tensor.matmul(kh_ps, wk_r[:, c, ts(oc, 128)],
                                         k_c(c), start=(c == 0),
                                         stop=(c == 3), skip_group_check=True)
                    nc.scalar.copy(kh_sb[:, oc, :], kh_ps)
                    nc.vector.tensor_mul(kh2_r[:, oc, :], kh_sb[:, oc, :],
                                         kh_sb[:, oc, :])
                nc.sync.dma_start(
                    out=kh_out[:, :].rearrange("(c p) f -> p c f", p=128),
                    in_=kh_sb)

                # local stabilizer max over heads/positions of -0.5||kh_h||^2;
                # each 128-partition oc chunk holds 2 heads (64 dims each)
                hkm = statp.tile([2, 4], F32)
                for oc in range(4):
                    hk_ps = psb.tile([2, SLA], F32, tag="hk", bufs=1,
                                     name=f"hk{oc}")
                    nc.tensor.matmul(hk_ps, nh_r, kh2_r[:, oc, :],
                                     start=True, stop=True,
                                     skip_group_check=True)
                    nc.vector.reduce_max(hkm[:, oc:oc + 1], hk_ps,
                                         axis=mybir.AxisListType.X)
                stab_sb = statp.tile([2, 1], F32)
                nc.vector.reduce_max(stab_sb, hkm, axis=mybir.AxisListType.X)
                nc.scalar.dma_start(out=stab_out[:, :], in_=stab_sb)

                # rstd broadcast to 128 partitions
                rsbc_sb = work.tile([128, SLA], BF16)
                rsbc_ps = psb.tile([128, SLA], F32, tag="rsbc", bufs=1)
                nc.tensor.matmul(rsbc_ps, gq_sb[0:1, D_MODEL:D_MODEL + 128],
                                 rstd_r, start=True, stop=True,
                                 skip_group_check=True)
                nc.scalar.copy(rsbc_sb, rsbc_ps)

                for oc in range(4):
                    qh_ps = psb.tile([128, SLA], F32, tag="qh")
                    for c in range(4):
                        nc.tensor.matmul(qh_ps, wq_r[:, c, ts(oc, 128)],
                                         q_c(c), start=(c == 0),
                                         stop=False, skip_group_check=True)
                    nc.tensor.matmul(qh_ps, gq_sb[0:1, ts(oc, 128)], mu_r,
                                     start=False, stop=True,
                                     skip_group_check=True)
                    nc.vector.tensor_mul(qh_sb[:, oc, :], qh_ps, rsbc_sb)
                    nc.scalar.activation(qh_sb[:, oc, :], qh_sb[:, oc, :], IDENT,
                                         bias=cq_sb[:, oc:oc + 1], scale=1.0)
                nc.sync.dma_start(
                    out=qh_out[:, :].rearrange("(c p) f -> p c f", p=128),
                    in_=qh_sb)

                for sc in range(2):
                    vh_ps = psb.tile([128, D_MODEL], F32, tag="vh")
                    for c in range(4):
                        nc.tensor.matmul(vh_ps, v_c(c)[:, ts(sc, 128)],
                                         wv_r[:, c, :], start=(c == 0),
                                         stop=(c == 3), skip_group_check=True)
                    if sc == 0:
                        nc.scalar.copy(vhT_sb[:, sc, :], vh_ps)
                    else:
                        nc.vector.tensor_copy(vhT_sb[:, sc, :], vh_ps)
                nc.sync.dma_start(
                    out=vhT_out[:, :].rearrange("(s p) f -> p s f", p=128),
                    in_=vhT_sb)
    nc.compile()
    return nc


# --------------------------------------------------------------------------
# Phase B: head-sharded Performer attention + FC row-block
# --------------------------------------------------------------------------
def build_phase_b(debug=False):
    nc = bacc.Bacc("TRN2", target_bir_lowering=False, debug=False, num_devices=NC)
    khh = nc.dram_tensor("khh", [D_K, N], BF16, kind="ExternalInput")
    qhh = nc.dram_tensor("qhh", [D_K, N], BF16, kind="ExternalInput")
    vht = nc.dram_tensor("vht", [128, NCH * D_V], BF16, kind="ExternalInput")
    # bf16 const blobs: blob1 = rft|rneg (needed first), blob2 = idm|tri|wfc
    NB1 = 2 * M
    NB2 = 128 + 128 + D_MODEL
    blob1 = nc.dram_tensor("blob1", [D_K, NB1], BF16, kind="ExternalInput")
    blob2 = nc.dram_tensor("blob2", [128, NB2], BF16, kind="ExternalInput")
    # f32 consts [128, 129]: col0 = -k_stab, cols 1:129 identity (transpose)
    stabc = nc.dram_tensor("stabc", [128, 129], F32, kind="ExternalInput")
    out_d = nc.dram_tensor("out", [N, D_MODEL], BF16, kind="ExternalOutput")

    with tile.TileContext(nc) as tc:
        with (
            tc.tile_pool(name="consts", bufs=1) as consts,
            tc.tile_pool(name="krows", bufs=1) as krows,
            tc.tile_pool(name="feat", bufs=1) as feat,
            tc.tile_pool(name="ktrp", bufs=1) as ktrp,
            tc.tile_pool(name="ktT", bufs=NCH) as ktTp,
            tc.tile_pool(name="atp", bufs=NCH) as atp,
            tc.tile_pool(name="ssb", bufs=NCH + 1) as ssbp,
            tc.tile_pool(name="post", bufs=4) as post,
            tc.tile_pool(name="outp", bufs=3) as outp,
        ):
            # ---- loads (order: what the k/q feature path needs first) ----
            b1_sb = consts.tile([D_K, NB1], BF16)
            nc.scalar.dma_start(out=b1_sb, in_=blob1[:, :])
            stab_full = consts.tile([128, 129], F32)
            nc.scalar.dma_start(out=stab_full, in_=stabc[:, :])
            stab_sb = stab_full[:, 0:1]
            idf_r = stab_full[:, 1:129]
            khr = krows.tile([D_K, N], BF16)
            nc.sync.dma_start(out=khr, in_=khh[:, :])
            qhr = krows.tile([D_K, N], BF16)
            nc.sync.dma_start(out=qhr, in_=qhh[:, :])
            b2_sb = consts.tile([128, NB2], BF16)
            nc.scalar.dma_start(out=b2_sb, in_=blob2[:, :])
            vha = krows.tile([128, NCH, 66], BF16)
            nc.gpsimd.dma_start(
                out=vha[:, :, 0:D_V],
                in_=vht[:, :].rearrange("p (ch f) -> p ch f", ch=NCH))

            rft_r = b1_sb[:, 0:M]
            rneg_r = b1_sb[:, M:2 * M]
            id_r = b2_sb[:, 0:128]
            tri_r = b2_sb[:, 128:256]
            wfc_r = b2_sb[0:D_V, 256:256 + D_MODEL]

            # vha constant columns: 64 -> 1.0, 65 -> 0.0
            nc.gpsimd.memset(vha[:, :, D_V:D_V + 1], 1.0)
            nc.gpsimd.memset(vha[:, :, D_V + 1:D_V + 2], 0.0)

            # kh^2 rows
            kh2r = krows.tile([D_K, N], BF16)
            nc.vector.tensor_mul(kh2r, khr, khr)

            # q~ features m-major [128, 3, N]; mc2 = rows 0..9
            qt_feat = feat.tile([128, 3, N], BF16)
            # k~ features seq-major [C, ch, 266]
            ktr = ktrp.tile([128, NCH, M], BF16)

            s_tiles = [ssbp.tile([128, 3, 66], BF16, tag="ssb", name=f"ssb{i}")
                       for i in range(NCH + 1)]

            ktT_tiles = {}
            at_list = []
            with (
                tc.tile_pool(name="psq", bufs=2, space="PSUM") as psq,
                tc.tile_pool(name="pskp", bufs=2, space="PSUM") as pskp,
                tc.tile_pool(name="pstr", bufs=2, space="PSUM") as pstr,
                tc.tile_pool(name="psat", bufs=1, space="PSUM") as psat,
                tc.tile_pool(name="pssd", bufs=1, space="PSUM") as pssd,
            ):
              def emit_kp(ch):
                    # k-feature projection for chunk ch (prefetched one chunk
                    # ahead so the PE isn't idle while ACT runs the exp)
                    kp_ps = pskp.tile([C, M], F32, tag="kp", name=f"kp{ch}")
                    nc.tensor.matmul(kp_ps, khr[:, ts(ch, C)], rft_r,
                                     start=True, stop=False,
                                     skip_group_check=True)
                    nc.tensor.matmul(kp_ps, kh2r[:, ts(ch, C)], rneg_r,
                                     start=False, stop=True,
                                     skip_group_check=True)
                    with nc.allow_low_precision(reason="bf16 features"):
                        nc.scalar.activation(
                            ktr[:, ch, 0:M], kp_ps, EXP,
                            bias=stab_sb, scale=1.0)

              for s in range(NSL):
                    # ---- q features for slice s ----
                    for mc in range(3):
                        mrows = 128 if mc < 2 else 10
                        qp_ps = psq.tile([128, SL], F32, tag="qp")
                        nc.tensor.matmul(
                            qp_ps[0:mrows, :], rft_r[:, ds(mc * 128, mrows)],
                            qhr[:, ts(s, SL)], start=True, stop=True,
                            skip_group_check=True)
                        nc.scalar.activation(
                            qt_feat[0:mrows, mc, ts(s, SL)],
                            qp_ps[0:mrows, :], EXP, bias=0.0, scale=1.0)

                    # ---- k features + scan for the 4 chunks of slice s ----
                    for ch in range(4 * s, 4 * s + 4):
                        if ch == 0:
                            emit_kp(0)
                        if ch + 1 < NCH:
                            emit_kp(ch + 1)
                        # transpose k~ chunk to m-major
                        tp_ps = pstr.tile([128, 3, 128], BF16, tag="tp")
                        nc.tensor.transpose(tp_ps[:, 0, :], ktr[:, ch, 0:128], id_r)
                        nc.tensor.transpose(tp_ps[:, 1, :], ktr[:, ch, 128:256], id_r)
                        nc.tensor.transpose(tp_ps[0:10, 2, :], ktr[:, ch, 256:266], id_r)
                        ktT = ktTp.tile([128, 3, C], BF16, tag="ktT", name=f"ktT{ch}")
                        nc.scalar.copy(ktT[:, 0, :], tp_ps[:, 0, :])
                        # rows 10.. of the mc2 block are never read; copying
                        # them (uninitialized) is harmless and merges two
                        # copies into one
                        nc.vector.tensor_copy(ktT[:, 1:3, :], tp_ps[:, 1:3, :])
                        ktT_tiles[ch] = ktT

                        # diagonal score block (keys ch x queries ch), masked
                        at_ps = psat.tile([C, C], F32, tag="at")
                        for mc in range(2):
                            nc.tensor.matmul(at_ps, ktT[:, mc, :],
                                             qt_feat[:, mc, ts(ch, C)],
                                             start=(mc == 0), stop=False,
                                             skip_group_check=True)
                        nc.tensor.matmul(at_ps, ktT[0:10, 2, :],
                                         qt_feat[0:10, 2, ts(ch, C)],
                                         start=False, stop=True,
                                         skip_group_check=True)
                        at_r = atp.tile([C, C], BF16, tag="at_r", name=f"atr{ch}")
                        nc.vector.tensor_mul(at_r, at_ps, tri_r)
                        at_list.append(at_r)

                        # state update for this chunk
                        sd_ps = pssd.tile([128, 3, 66], F32, tag="sd")
                        if ch == 0:
                            # one-time zero of mc2 rows the matmuls never
                            # write; persists across psum reuse (bufs=1)
                            nc.vector.memset(sd_ps[:, 2, :], 0.0)
                        for mc, cols in ((0, (0, 128)), (1, (128, 256)),
                                         (2, (256, 266))):
                            dst = sd_ps[:, mc, :] if mc < 2 else sd_ps[0:10, 2, :]
                            nc.tensor.matmul(dst, ktr[:, ch, cols[0]:cols[1]],
                                             vha[:, ch, 0:66], start=True,
                                             stop=True, skip_group_check=True)
                        if ch == 0:
                            nc.vector.tensor_copy(s_tiles[1][:, :, :], sd_ps)
                        else:
                            nc.vector.tensor_add(s_tiles[ch + 1][:, :, :],
                                                 s_tiles[ch][:, :, :], sd_ps)

            # z column = final state's column 64
            zcol = feat.tile([128, 3, 1], BF16)
            nc.vector.tensor_copy(zcol, s_tiles[NCH][:, :, 64:65])

            # ---- per-chunk output, normalize, FC ----
            with (
                tc.tile_pool(name="pso", bufs=3, space="PSUM") as pso,
                tc.tile_pool(name="psfc", bufs=2, space="PSUM") as psfc,
                tc.tile_pool(name="pstr2", bufs=2, space="PSUM") as pstr2,
            ):
                for ch in range(NCH):
                    o_ps = pso.tile([C, 67], F32, tag="o")
                    if ch > 0:
                        for mc in range(2):
                            nc.tensor.matmul(o_ps[:, 0:66],
                                             qt_feat[:, mc, ts(ch, C)],
                                             s_tiles[ch][:, mc, 0:66],
                                             start=(mc == 0), stop=False,
                                             skip_group_check=True)
                        nc.tensor.matmul(o_ps[:, 0:66],
                                         qt_feat[0:10, 2, ts(ch, C)],
                                         s_tiles[ch][0:10, 2, 0:66],
                                         start=False, stop=False,
                                         skip_group_check=True)
                        nc.tensor.matmul(o_ps[:, 0:66], at_list[ch],
                                         vha[:, ch, :], start=False, stop=True,
                                         skip_group_check=True)
                    else:
                        nc.tensor.matmul(o_ps[:, 0:66], at_list[ch],
                                         vha[:, ch, :], start=True, stop=True,
                                         skip_group_check=True)
                    # d = q~ . z in column 66
                    for mc in range(2):
                        nc.tensor.matmul(o_ps[:, 66:67],
                                         qt_feat[:, mc, ts(ch, C)],
                                         zcol[:, mc, :], start=(mc == 0),
                                         stop=False, skip_group_check=True)
                    nc.tensor.matmul(o_ps[:, 66:67], qt_feat[0:10, 2, ts(ch, C)],
                                     zcol[0:10, 2, :], start=False, stop=True,
                                     skip_group_check=True)
                    # normalization tail kept on one engine (program-order
                    # chaining, no cross-engine semaphore hops)
                    dcols = post.tile([C, 3], F32, tag="dcols")
                    nc.scalar.copy(dcols, o_ps[:, 64:67])
                    dd = post.tile([C, 1], F32, tag="dd")
                    nc.gpsimd.tensor_mul(dd, dcols[:, 0:1], dcols[:, 2:3])
                    rec = post.tile([C, 1], F32, tag="rec")
                    nc.vector.reciprocal(rec, dd)
                    attn_r = post.tile([C, D_V], F32, tag="attn")
                    nc.vector.tensor_scalar_mul(attn_r, o_ps[:, 0:D_V], rec)
                    tr_ps = pstr2.tile([D_V, C], F32, tag="tr")
                    nc.tensor.transpose(tr_ps, attn_r, idf_r)
                    attnT_r = post.tile([D_V, C], BF16, tag="attnT")
                    (nc.scalar.copy if ch % 2 else nc.vector.tensor_copy)(
                        attnT_r, tr_ps)
                    fc_ps = psfc.tile([C, D_MODEL], F32, tag="fc")
                    nc.tensor.matmul(fc_ps, attnT_r, wfc_r, start=True, stop=True,
                                     skip_group_check=True)
                    o_sb = outp.tile([C, D_MODEL], BF16, tag="osb",
                                     name=f"osb{ch}")
                    eng = (nc.scalar.copy, nc.vector.tensor_copy)[ch % 2]
                    eng(o_sb, fc_ps)
                    nc.sync.dma_start(out=out_d[ts(ch, C), :], in_=o_sb)
    nc.compile()
    return nc


# --------------------------------------------------------------------------
# Host orchestration
# --------------------------------------------------------------------------
_CACHE = {}


def _get_programs():
    if "a" not in _CACHE:
        _CACHE["a"] = build_phase_a()
        _CACHE["b"] = build_phase_b()
    return _CACHE["a"], _CACHE["b"]


def _bf(x):
    return np.ascontiguousarray(np.asarray(x, np.float32).astype(NPBF))


def _prep_a_maps(q, k, v, Wq, Wk, Wv, gamma, beta):
    qT = np.ascontiguousarray(q[0].T)
    kT = np.ascontiguousarray(k[0].T)
    vT = np.ascontiguousarray(v[0].T)
    Wqe = (gamma[:, None] * Wq) * SCALE
    Wke = Wk * SCALE
    cq_all = (beta @ Wq) * SCALE                       # [512]
    cqm = np.zeros((128, 5), np.float32)
    cqm[:, 0:4] = cq_all.reshape(4, 128).T
    cqm[0, 4] = LN_EPS
    cb = np.zeros((128, 4), np.float32)
    cb[:, 0] = 1.0 / D_MODEL
    cb[:, 1] = 1.0
    cb[0:64, 2] = -0.5
    cb[64:128, 3] = -0.5
    gqon = np.concatenate([-Wqe.sum(axis=0),
                           np.ones(128, np.float32)])[None, :]
    Wqe_b, Wke_b, Wv_b, cb_b, gq_b = (_bf(Wqe), _bf(Wke), _bf(Wv), _bf(cb),
                                      _bf(gqon))
    in_a = []
    for j in range(NC):
        sl = slice(j * SLA, (j + 1) * SLA)
        xs = np.concatenate([qT[:, sl], kT[:, sl], vT[:, sl]], axis=1)
        in_a.append({
            "xs": _bf(xs),
            "Wqe": Wqe_b, "Wke": Wke_b, "Wv": Wv_b,
            "cb": cb_b, "gqon": gq_b, "cqm": cqm,
        })
    return in_a


def _prep_b_maps(W_fc, rf, res_a):
    kh_full = np.concatenate([np.asarray(r["kh"]) for r in res_a], axis=1)
    qh_full = np.concatenate([np.asarray(r["qh"]) for r in res_a], axis=1)
    vh_full = np.concatenate([np.asarray(r["vhT"]) for r in res_a], axis=0)
    k_stab = np.float32(max(float(np.max(r["stab"])) for r in res_a))

    rftT = rf.T.astype(np.float32)                     # [64, 266]
    tri = np.triu(np.ones((C, C), np.float32))
    identm = np.eye(128, dtype=np.float32)
    stabc = np.concatenate([np.full((128, 1), -k_stab, np.float32),
                            identm], axis=1)

    in_b = []
    for h in range(NC):
        rows = slice(h * D_K, (h + 1) * D_K)
        vh_h = np.asarray(vh_full[:, h * D_V:(h + 1) * D_V], np.float32)
        vht = vh_h.reshape(NCH, 128, D_V).transpose(1, 0, 2).reshape(
            128, NCH * D_V)
        blob1 = np.concatenate([rftT, np.full((D_K, M), -0.5, np.float32)],
                               axis=1)
        blob2 = np.zeros((128, 128 + 128 + D_MODEL), np.float32)
        blob2[:, 0:128] = identm
        blob2[:, 128:256] = tri
        blob2[0:D_K, 256:256 + D_MODEL] = W_fc[rows, :] * float(M)
        in_b.append({
            "khh": _bf(kh_full[rows]),
            "qhh": _bf(qh_full[rows]),
            "vht": _bf(vht),
            "blob1": _bf(blob1),
            "blob2": _bf(blob2),
            "stabc": stabc,
        })
    return in_b


def _cast_all(*arrs):
    return [np.asarray(a, np.float32) for a in arrs]


def kernel(q, k, v, Wq, Wk, Wv, W_fc, b_fc, gamma, beta, rf):
    q, k, v, Wq, Wk, Wv, W_fc, b_fc, gamma, beta, rf = _cast_all(
        q, k, v, Wq, Wk, Wv, W_fc, b_fc, gamma, beta, rf)

    nc_a, nc_b = _get_programs()
    cores = list(range(NC))

    in_a = _prep_a_maps(q, k, v, Wq, Wk, Wv, gamma, beta)
    res_a = run_bass_kernel_spmd(nc_a, in_a, core_ids=cores)

    in_b = _prep_b_maps(W_fc, rf, res_a.results)
    res_b = run_bass_kernel_spmd(nc_b, in_b, core_ids=cores)

    out = np.zeros((N, D_MODEL), np.float32)
    for r in res_b.results:
        out += np.asarray(r["out"], np.float32)
    out += b_fc[None, :]
    out += q[0]
    return out[None].astype(np.float32)


def trace_args(inputs):
    """For test.py: returns [(phase, nc, in_maps), ...] re-runnable with trace."""
    q, k, v, Wq, Wk, Wv, W_fc, b_fc, gamma, beta, rf = _cast_all(
        inputs["q"], inputs["k"], inputs["v"], inputs["Wq"], inputs["Wk"],
        inputs["Wv"], inputs["W_fc"], inputs["b_fc"], inputs["gamma"],
        inputs["beta"], inputs["rf"])
    nc_a, nc_b = _get_programs()
    in_a = _prep_a_maps(q, k, v, Wq, Wk, Wv, gamma, beta)
    res_a = run_bass_kernel_spmd(nc_a, in_a, core_ids=list(range(NC)))
    in_b = _prep_b_maps(W_fc, rf, res_a.results)
    return [("a", nc_a, in_a), ("b", nc_b, in_b)]


if __name__ == "__main__":
    rng = np.random.default_rng(0)
    inputs = {
        "q": rng.standard_normal((1, N, D_MODEL)).astype(np.float32),
        "k": rng.standard_normal((1, N, D_MODEL)).astype(np.float32),
        "v": rng.standard_normal((1, N, D_MODEL)).astype(np.float32),
        "Wq": (rng.standard_normal((D_MODEL, 512)) * 0.04).astype(np.float32),
        "Wk": (rng.standard_normal((D_MODEL, 512)) * 0.04).astype(np.float32),
        "Wv": (rng.standard_normal((D_MODEL, 512)) * 0.04).astype(np.float32),
        "W_fc": (rng.standard_normal((512, D_MODEL)) * 0.04).astype(np.float32),
        "b_fc": np.zeros(D_MODEL, np.float32),
        "gamma": np.ones(D_MODEL, np.float32),
        "beta": np.zeros(D_MODEL, np.float32),
        "rf": rng.standard_normal((M, D_K)).astype(np.float32),
    }
    out = kernel(**inputs)
    print("kernel output", out.shape, out.dtype)
